# revision 3
# baseline (speedup 1.0000x reference)
"""MLA (Multi-Head Latent Attention) Bass kernel for 8 Trainium2 NeuronCores.

Sharding: 8 cores = 2 (batch) x 4 (head groups). Core c -> batch c//4,
group g=c%4 owning heads {2g, 2g+1, 2g+8, 2g+9} (paired h/h+8 so the
rotate-half RoPE over d_model=2048 stays core-local).

All activations flow on-device in transposed [feature, token] layout so no
on-chip transposes are needed (the host pre-transposes x). Attention scores
are computed in [k, q] layout; the softmax denominator is computed with an
all-ones matmul on the PE (scores are bounded, so no max subtraction), exp
runs on the scalar engine straight out of PSUM, and 1/denom is folded into
the attention-output scaling. Matmuls use fp32r (full PE rate at N=512).

Each core computes a partial out^T = (attn_out_g @ Wout[rows_g]).T for its
4 heads; the host sums the 4 partials per batch and transposes. bout is
added on-device by the g==0 cores only.
"""
import os
import sys

if "/opt/trn_rl_repo" not in sys.path:
    sys.path.insert(0, "/opt/trn_rl_repo")

import numpy as np

D_MODEL = 2048
Q_LAT = 1536
KV_LAT = 512
NUM_HEADS = 16
HD = 128
B, S = 2, 2048
SCALE = 1.0 / np.sqrt(2.0 * HD)  # 1/16

QT = 512          # query tile width (matmul free dim)
NQT = S // QT     # 4
NC_DM = D_MODEL // 128   # 16 chunks of the model dim
NC_QL = Q_LAT // 128     # 12
NC_KV = KV_LAT // 128    # 4
NKC = S // 128           # 16 key chunks

_CACHE = {}
LAST_RESULT = None


def _strip_cols(g):
    """Global column ranges (width 128) of the 4 local head strips, in local
    order [2g, 2g+1, 2g+8, 2g+9]."""
    return [256 * g, 256 * g + 128, 1024 + 256 * g, 1024 + 256 * g + 128]


def _build_bass():
    import concourse.bass as bass
    from concourse import bacc, mybir
    from concourse.tile import TileContext

    f32 = mybir.dt.float32
    f32r = mybir.dt.float32r
    AF = mybir.ActivationFunctionType

    nc = bacc.Bacc("TRN2", target_bir_lowering=False, debug=False)

    def inp(name, shape, dt=None):
        return nc.dram_tensor(name, list(shape), dt or f32r, kind="ExternalInput")

    xqT = inp("xqT", (D_MODEL, S))
    xkT = inp("xkT", (D_MODEL, S))
    wq_down = inp("wq_down", (NC_QL, 128, NC_DM * 128))     # [s][p][c*128+f]
    wkv_down = inp("wkv_down", (NC_KV, 128, NC_DM * 128))
    wk_rope = inp("wk_rope", (128, NC_DM * 128))            # [p][c*128+f]
    wq_up = inp("wq_up", (4, 128, NC_QL * 128))             # [strip][p][c*128+f]
    wq_rope = inp("wq_rope", (4, 128, NC_QL * 128))
    wk_up = inp("wk_up", (4, 128, NC_KV * 128))
    wv_up = inp("wv_up", (128, NC_KV * 512))                # [p][c*512+f]
    wout = inp("wout", (128, 64 * 128))                     # [p][(m*4+h)*128+f]
    cos_q = inp("cos_q", (2, 128, S), f32)                       # [block j][d][q]
    sin_q = inp("sin_q", (2, 128, S), f32)
    cos_k = inp("cos_k", (64, S), f32)
    sin_k = inp("sin_k", (64, S), f32)
    masks = inp("masks", (128, 4 * QT))                     # [kl][(o*QT)+ql]
    ones = inp("ones", (128, 128))
    bias = inp("bias", (128, NC_DM), f32)                        # [p][m]

    outT = nc.dram_tensor("outT", [D_MODEL, S], f32, kind="ExternalOutput")

    # DRAM scratch for inter-phase spills
    latq_d = nc.dram_tensor("latq_d", [NC_QL, 128, S], f32r, kind="Internal")
    qnew_d = nc.dram_tensor("qnew_d", [8, 128, S], f32r, kind="Internal")
    kproj_d = nc.dram_tensor("kproj_d", [4, 128, S], f32r, kind="Internal")
    krope_d = nc.dram_tensor("krope_d", [128, S], f32r, kind="Internal")
    v_d = nc.dram_tensor("v_d", [NKC, 128, 512], f32r, kind="Internal")

    def r(ap):
        return ap

    xqT_v = xqT.ap().rearrange("(c p) q -> p c q", p=128)   # [128, 16, 2048]
    xkT_v = xkT.ap().rearrange("(c p) q -> p c q", p=128)

    with TileContext(nc, pool_alloc_mode="queue") as tc:
        # ------------- Phase 1: K/V build (latkv, k_proj, V, k_rope) ----
        with tc.tile_pool(name="p1w", bufs=1) as p1w, \
             tc.tile_pool(name="p1x", bufs=2) as p1x, \
             tc.tile_pool(name="p1l", bufs=1) as p1l, \
             tc.tile_pool(name="p1t", bufs=2) as p1t, \
             tc.tile_pool(name="p1ps", bufs=2, space="PSUM") as p1ps:
            wkv_sb = p1w.tile([128, NC_KV * NC_DM * 128], f32r)
            for s in range(NC_KV):
                nc.sync.dma_start(
                    out=wkv_sb[:, s * NC_DM * 128:(s + 1) * NC_DM * 128],
                    in_=wkv_down.ap()[s])
            wkr_sb = p1w.tile([128, NC_DM * 128], f32r)
            nc.sync.dma_start(out=wkr_sb, in_=wk_rope.ap())
            wku_sb = p1w.tile([128, 4 * NC_KV * 128], f32r)
            for s in range(4):
                nc.sync.dma_start(
                    out=wku_sb[:, s * NC_KV * 128:(s + 1) * NC_KV * 128],
                    in_=wk_up.ap()[s])
            wvu_sb = p1w.tile([128, NC_KV * 512], f32r)
            nc.sync.dma_start(out=wvu_sb, in_=wv_up.ap())
            cosk_sb = p1w.tile([64, S], f32)
            sink_sb = p1w.tile([64, S], f32)
            nc.sync.dma_start(out=cosk_sb, in_=cos_k.ap())
            nc.sync.dma_start(out=sink_sb, in_=sin_k.ap())

            for kh in range(2):  # k halves of 1024
                k0 = kh * 1024
                latkv = p1l.tile([128, NC_KV, 1024], f32r, tag="latkv")
                krraw = p1l.tile([128, 1024], f32, tag="krraw")
                for kt in range(2):  # two 512-tiles within the half
                    kk = k0 + kt * QT
                    xk_t = p1x.tile([128, NC_DM, QT], f32r, tag="xk")
                    nc.sync.dma_start(out=xk_t, in_=xkT_v[:, :, kk:kk + QT])
                    for s in range(NC_KV):
                        ps = p1ps.tile([128, QT], f32, tag="ps")
                        for c in range(NC_DM):
                            nc.tensor.matmul(
                                ps, r(wkv_sb[:, (s * NC_DM + c) * 128:(s * NC_DM + c + 1) * 128]),
                                r(xk_t[:, c, :]), start=(c == 0), stop=(c == NC_DM - 1))
                        nc.scalar.copy(out=latkv[:, s, kt * QT:(kt + 1) * QT], in_=ps)
                    ps = p1ps.tile([128, QT], f32, tag="ps")
                    for c in range(NC_DM):
                        nc.tensor.matmul(
                            ps, r(wkr_sb[:, c * 128:(c + 1) * 128]),
                            r(xk_t[:, c, :]), start=(c == 0), stop=(c == NC_DM - 1))
                    nc.scalar.copy(out=krraw[:, kt * QT:(kt + 1) * QT], in_=ps)

                # k_rope combine for this half
                krb = p1t.tile([64, 1024], f32, tag="krb")
                nc.sync.dma_start(out=krb, in_=krraw[64:128, :])
                ck = cosk_sb[:, k0:k0 + 1024]
                sk = sink_sb[:, k0:k0 + 1024]
                t1 = p1t.tile([64, 1024], f32, tag="krt1")
                t2 = p1t.tile([64, 1024], f32, tag="krt2")
                otop = p1t.tile([64, 1024], f32r, tag="krot")
                obot = p1t.tile([64, 1024], f32r, tag="krob")
                nc.vector.tensor_mul(t1, krraw[0:64, :], ck)
                nc.vector.tensor_mul(t2, krb, sk)
                nc.vector.tensor_sub(otop, t1, t2)
                nc.sync.dma_start(out=krope_d.ap()[0:64, k0:k0 + 1024], in_=otop)
                nc.vector.tensor_mul(t1, krb, ck)
                nc.vector.tensor_mul(t2, krraw[0:64, :], sk)
                nc.vector.tensor_add(obot, t1, t2)
                nc.sync.dma_start(out=krope_d.ap()[64:128, k0:k0 + 1024], in_=obot)

                # k_projT strips for this half
                for s in range(4):
                    for kt in range(2):
                        ps = p1ps.tile([128, QT], f32, tag="ps")
                        for c in range(NC_KV):
                            nc.tensor.matmul(
                                ps, r(wku_sb[:, (s * NC_KV + c) * 128:(s * NC_KV + c + 1) * 128]),
                                r(latkv[:, c, kt * QT:(kt + 1) * QT]),
                                start=(c == 0), stop=(c == NC_KV - 1))
                        cp = p1t.tile([128, QT], f32r, tag="kpcp")
                        nc.scalar.copy(out=cp, in_=ps)
                        nc.sync.dma_start(
                            out=kproj_d.ap()[s][:, k0 + kt * QT:k0 + (kt + 1) * QT],
                            in_=cp)
                # V natural for this half
                for kc in range(8):  # 128-chunks within the half
                    ps = p1ps.tile([128, 512], f32, tag="ps")
                    for c in range(NC_KV):
                        nc.tensor.matmul(
                            ps, r(latkv[:, c, kc * 128:(kc + 1) * 128]),
                            r(wvu_sb[:, c * 512:(c + 1) * 512]),
                            start=(c == 0), stop=(c == NC_KV - 1))
                    cp = p1t.tile([128, 512], f32r, tag="vcp")
                    nc.scalar.copy(out=cp, in_=ps)
                    nc.sync.dma_start(out=v_d.ap()[kh * 8 + kc], in_=cp)

        # ---------------- Phase 2: latqT = (xq @ Wq_down)^T -------------
        with tc.tile_pool(name="p2w", bufs=1) as p2w, \
             tc.tile_pool(name="p2x", bufs=2) as p2x, \
             tc.tile_pool(name="p2c", bufs=3) as p2c, \
             tc.tile_pool(name="p2ps", bufs=2, space="PSUM") as p2ps:
            wqd_sb = p2w.tile([128, NC_QL * NC_DM * 128], f32r)  # 96KB/part
            for s in range(NC_QL):
                nc.sync.dma_start(
                    out=wqd_sb[:, s * NC_DM * 128:(s + 1) * NC_DM * 128],
                    in_=wq_down.ap()[s])
            for qt in range(NQT):
                xq_t = p2x.tile([128, NC_DM, QT], f32r, tag="xq")
                nc.sync.dma_start(out=xq_t, in_=xqT_v[:, :, qt * QT:(qt + 1) * QT])
                for s in range(NC_QL):
                    ps = p2ps.tile([128, QT], f32, tag="ps")
                    for c in range(NC_DM):
                        nc.tensor.matmul(
                            ps, r(wqd_sb[:, (s * NC_DM + c) * 128:(s * NC_DM + c + 1) * 128]),
                            r(xq_t[:, c, :]), start=(c == 0), stop=(c == NC_DM - 1))
                    cp = p2c.tile([128, QT], f32r, tag="cp")
                    nc.scalar.copy(out=cp, in_=ps)
                    nc.sync.dma_start(
                        out=latq_d.ap()[s][:, qt * QT:(qt + 1) * QT], in_=cp)

        # ------------- Phase 3: q_projT + ropeT per head strip ----------
        with tc.tile_pool(name="p3w", bufs=1) as p3w, \
             tc.tile_pool(name="p3l", bufs=2) as p3l, \
             tc.tile_pool(name="p3t", bufs=2) as p3t, \
             tc.tile_pool(name="p3ps", bufs=2, space="PSUM") as p3ps:
            wqu_sb = p3w.tile([128, 4 * NC_QL * 128], f32r)
            wqr_sb = p3w.tile([128, 4 * NC_QL * 128], f32r)
            cosq_sb = p3w.tile([128, 2, S], f32)
            sinq_sb = p3w.tile([128, 2, S], f32)
            for s in range(4):
                nc.sync.dma_start(
                    out=wqu_sb[:, s * NC_QL * 128:(s + 1) * NC_QL * 128],
                    in_=wq_up.ap()[s])
                nc.sync.dma_start(
                    out=wqr_sb[:, s * NC_QL * 128:(s + 1) * NC_QL * 128],
                    in_=wq_rope.ap()[s])
            for j in range(2):
                nc.sync.dma_start(out=cosq_sb[:, j, :], in_=cos_q.ap()[j])
                nc.sync.dma_start(out=sinq_sb[:, j, :], in_=sin_q.ap()[j])

            for qt in range(NQT):
                q0 = qt * QT
                lat_t = p3l.tile([128, NC_QL, QT], f32r, tag="lat")
                nc.sync.dma_start(
                    out=lat_t,
                    in_=latq_d.ap().rearrange("s p q -> p s q")[:, :, q0:q0 + QT])
                raw = []
                for s in range(4):
                    # q_proj strip
                    ps = p3ps.tile([128, QT], f32, tag="ps")
                    for c in range(NC_QL):
                        nc.tensor.matmul(
                            ps, r(wqu_sb[:, (s * NC_QL + c) * 128:(s * NC_QL + c + 1) * 128]),
                            r(lat_t[:, c, :]), start=(c == 0), stop=(c == NC_QL - 1))
                    cp = p3t.tile([128, QT], f32r, tag=f"qp{s}")
                    nc.scalar.copy(out=cp, in_=ps)
                    nc.sync.dma_start(out=qnew_d.ap()[2 * s][:, q0:q0 + QT], in_=cp)
                    # q_rope raw strip
                    ps2 = p3ps.tile([128, QT], f32, tag="ps")
                    for c in range(NC_QL):
                        nc.tensor.matmul(
                            ps2, r(wqr_sb[:, (s * NC_QL + c) * 128:(s * NC_QL + c + 1) * 128]),
                            r(lat_t[:, c, :]), start=(c == 0), stop=(c == NC_QL - 1))
                    rw = p3t.tile([128, QT], f32, tag=f"raw{s}")
                    nc.scalar.copy(out=rw, in_=ps2)
                    raw.append(rw)
                # rope combine: strips 0,1 = block A (j=0,1); 2,3 = block B
                for j in range(2):
                    a, b = raw[j], raw[2 + j]
                    cj = cosq_sb[:, j, q0:q0 + QT]
                    sj = sinq_sb[:, j, q0:q0 + QT]
                    t1 = p3t.tile([128, QT], f32, tag=f"t1{j}")
                    t2 = p3t.tile([128, QT], f32, tag=f"t2{j}")
                    outa = p3t.tile([128, QT], f32r, tag=f"oa{j}")
                    outb = p3t.tile([128, QT], f32r, tag=f"ob{j}")
                    nc.vector.tensor_mul(t1, a, cj)
                    nc.vector.tensor_mul(t2, b, sj)
                    nc.vector.tensor_sub(outa, t1, t2)
                    nc.sync.dma_start(out=qnew_d.ap()[2 * j + 1][:, q0:q0 + QT], in_=outa)
                    nc.vector.tensor_mul(t1, b, cj)
                    nc.vector.tensor_mul(t2, a, sj)
                    nc.vector.tensor_add(outb, t1, t2)
                    nc.sync.dma_start(out=qnew_d.ap()[2 * (2 + j) + 1][:, q0:q0 + QT], in_=outb)

        # ------------- Phase 4: attention + output projection -----------
        with tc.tile_pool(name="p4kv", bufs=1) as p4kv, \
             tc.tile_pool(name="p4w", bufs=1) as p4w, \
             tc.tile_pool(name="p4q", bufs=2) as p4q, \
             tc.tile_pool(name="p4e", bufs=4) as p4e, \
             tc.tile_pool(name="p4a", bufs=2) as p4a, \
             tc.tile_pool(name="p4o", bufs=2) as p4o, \
             tc.tile_pool(name="p4ps", bufs=2, space="PSUM") as p4ps, \
             tc.tile_pool(name="p4pd", bufs=2, space="PSUM") as p4pd, \
             tc.tile_pool(name="p4pv", bufs=2, space="PSUM") as p4pv, \
             tc.tile_pool(name="p4po", bufs=2, space="PSUM") as p4po:
            kproj_sb = p4kv.tile([128, 4, S], f32r)
            nc.sync.dma_start(out=kproj_sb, in_=kproj_d.ap().rearrange("s p k -> p s k"))
            krope_sb = p4kv.tile([128, S], f32r)
            nc.sync.dma_start(out=krope_sb, in_=krope_d.ap())
            v_sb = p4kv.tile([128, NKC, 512], f32r)
            nc.sync.dma_start(out=v_sb, in_=v_d.ap().rearrange("c p f -> p c f"))
            wout_sb = p4w.tile([128, 64 * 128], f32r)
            nc.sync.dma_start(out=wout_sb, in_=wout.ap())
            masks_sb = p4w.tile([128, 4 * QT], f32r)
            nc.sync.dma_start(out=masks_sb, in_=masks.ap())
            ones_sb = p4w.tile([128, 128], f32r)
            nc.sync.dma_start(out=ones_sb, in_=ones.ap())
            bias_sb = p4w.tile([128, NC_DM], f32)
            nc.sync.dma_start(out=bias_sb, in_=bias.ap())

            for qt in range(NQT):
                q0 = qt * QT
                K = (q0 + QT) // 128  # causal: chunks 0..K-1
                qn = p4q.tile([128, 8, QT], f32r, tag="qn")
                nc.sync.dma_start(
                    out=qn, in_=qnew_d.ap().rearrange("s p q -> p s q")[:, :, q0:q0 + QT])
                attn = p4a.tile([128, 4, QT], f32r, tag="attn")
                for h in range(4):
                    psd = p4pd.tile([128, QT], f32, tag="psd")
                    psv = p4pv.tile([128, QT], f32, tag="psv")
                    for kc in range(K):
                        pss = p4ps.tile([128, QT], f32, tag="pss")
                        nc.tensor.matmul(
                            pss, r(kproj_sb[:, h, kc * 128:(kc + 1) * 128]),
                            r(qn[:, 2 * h, :]), start=True, stop=False)
                        nc.tensor.matmul(
                            pss, r(krope_sb[:, kc * 128:(kc + 1) * 128]),
                            r(qn[:, 2 * h + 1, :]), start=False, stop=True)
                        ex = p4e.tile([128, QT], f32r, tag="ex")
                        nc.scalar.activation(out=ex, in_=pss, func=AF.Exp, scale=float(SCALE))
                        o = kc - q0 // 128
                        if o >= 0:  # diagonal chunk: apply causal mask
                            nc.vector.tensor_mul(ex, ex, masks_sb[:, o * QT:(o + 1) * QT])
                        nc.tensor.matmul(
                            psd, r(ones_sb), r(ex),
                            start=(kc == 0), stop=(kc == K - 1), skip_group_check=True)
                        nc.tensor.matmul(
                            psv, r(v_sb[:, kc, h * 128:(h + 1) * 128]), r(ex),
                            start=(kc == 0), stop=(kc == K - 1), skip_group_check=True)
                    rec = p4e.tile([128, QT], f32, tag="rec")
                    nc.vector.reciprocal_approx_fast(out=rec, in_=psd)
                    nc.vector.tensor_mul(attn[:, h, :], psv, rec)
                # output projection for this q tile
                for m in range(NC_DM):
                    pso = p4po.tile([128, QT], f32, tag="pso")
                    for h in range(4):
                        nc.tensor.matmul(
                            pso, r(wout_sb[:, (m * 4 + h) * 128:(m * 4 + h + 1) * 128]),
                            r(attn[:, h, :]), start=(h == 0), stop=(h == 3))
                    oc = p4o.tile([128, QT], f32, tag="oc")
                    nc.scalar.activation(
                        out=oc, in_=pso, func=AF.Identity,
                        bias=bias_sb[:, m:m + 1], scale=1.0)
                    nc.sync.dma_start(
                        out=outT.ap()[m * 128:(m + 1) * 128, q0:q0 + QT], in_=oc)

    nc.finalize()
    return nc


def _host_pack(inputs):
    """Build the 8 per-core input maps from the full inputs."""
    xq = np.ascontiguousarray(inputs["inputs_q"], dtype=np.float32)
    xk = np.ascontiguousarray(inputs["inputs_k"], dtype=np.float32)
    Wq_down = np.asarray(inputs["Wq_down"], dtype=np.float32)
    Wkv_down = np.asarray(inputs["Wkv_down"], dtype=np.float32)
    Wq_up = np.asarray(inputs["Wq_up"], dtype=np.float32)
    Wk_up = np.asarray(inputs["Wk_up"], dtype=np.float32)
    Wv_up = np.asarray(inputs["Wv_up"], dtype=np.float32)
    Wq_rope = np.asarray(inputs["Wq_rope"], dtype=np.float32)
    Wk_rope = np.asarray(inputs["Wk_rope"], dtype=np.float32)
    Wout = np.asarray(inputs["Wout"], dtype=np.float32)
    bout = np.asarray(inputs["bout"], dtype=np.float32)

    def pack_lhs(W, n_strips, strip_starts, nchunks):
        # -> [n_strips, 128, nchunks*128]: [s][p][c*128+f]
        out = np.empty((n_strips, 128, nchunks * 128), dtype=np.float32)
        for s in range(n_strips):
            blk = W[:, strip_starts[s]:strip_starts[s] + 128]  # [nchunks*128, 128]
            out[s] = blk.reshape(nchunks, 128, 128).transpose(1, 0, 2).reshape(128, -1)
        return out

    xqT = [np.ascontiguousarray(xq[b].T) for b in range(B)]
    xkT = [np.ascontiguousarray(xk[b].T) for b in range(B)]

    wq_down_p = pack_lhs(Wq_down, NC_QL, [128 * s for s in range(NC_QL)], NC_DM)
    wkv_down_p = pack_lhs(Wkv_down, NC_KV, [128 * s for s in range(NC_KV)], NC_DM)
    wk_rope_p = pack_lhs(Wk_rope, 1, [0], NC_DM)[0]

    # rope tables
    iq = np.arange(1024, dtype=np.float64)
    inv_q = 1.0 / (10000.0 ** (iq * 2.0 / D_MODEL))
    pos = np.arange(S, dtype=np.float64)
    ang_q = pos[:, None] * inv_q[None, :]          # [S, 1024]
    ik = np.arange(64, dtype=np.float64)
    inv_k = 1.0 / (10000.0 ** (ik * 2.0 / HD))
    ang_k = pos[:, None] * inv_k[None, :]          # [S, 64]
    cos_k = np.ascontiguousarray(np.cos(ang_k).T.astype(np.float32))  # [64, S]
    sin_k = np.ascontiguousarray(np.sin(ang_k).T.astype(np.float32))

    # causal diag masks [128, 4*QT]
    kl = np.arange(128)[:, None]
    ql = np.arange(QT)[None, :]
    masks = np.concatenate(
        [(kl + 128 * o <= ql).astype(np.float32) for o in range(4)], axis=1)
    masks = np.ascontiguousarray(masks)
    ones = np.ones((128, 128), dtype=np.float32)

    in_maps = []
    for c in range(8):
        b, g = divmod(c, 4)
        cols = _strip_cols(g)
        wq_up_p = pack_lhs(Wq_up, 4, cols, NC_QL)
        wq_rope_p = pack_lhs(Wq_rope, 4, cols, NC_QL)
        wk_up_p = pack_lhs(Wk_up, 4, cols, NC_KV)
        # wv_up: [128, nc_kv*512]; cols4 concatenated in local order
        cols4 = np.concatenate([np.arange(cs, cs + 128) for cs in cols])
        Wv_g = Wv_up[:, cols4]                      # [512, 512]
        wv_up_p = Wv_g.reshape(NC_KV, 128, 512).transpose(1, 0, 2).reshape(128, -1)
        # wout: rows for local heads; [128, 64*128] = [p][(m*4+h)*128+f]
        Wout_g = Wout[cols4, :].reshape(4, 128, NC_DM, 128)   # [h][p][m][f]
        wout_p = np.ascontiguousarray(
            Wout_g.transpose(1, 2, 0, 3).reshape(128, -1))    # [p][m,h,f]
        # cos/sin q for blocks j=0,1: global cols 256g+128j+d (<1024)
        cos_q_p = np.empty((2, 128, S), dtype=np.float32)
        sin_q_p = np.empty((2, 128, S), dtype=np.float32)
        for j in range(2):
            idx = 256 * g + 128 * j + np.arange(128)
            cos_q_p[j] = np.cos(ang_q[:, idx]).T
            sin_q_p[j] = np.sin(ang_q[:, idx]).T
        bias_p = (bout if g == 0 else np.zeros_like(bout)).reshape(NC_DM, 128)
        bias_p = np.ascontiguousarray(bias_p.T)     # [128, m]

        in_maps.append({
            "xqT": xqT[b], "xkT": xkT[b],
            "wq_down": wq_down_p, "wkv_down": wkv_down_p, "wk_rope": wk_rope_p,
            "wq_up": wq_up_p, "wq_rope": wq_rope_p, "wk_up": wk_up_p,
            "wv_up": np.ascontiguousarray(wv_up_p), "wout": wout_p,
            "cos_q": cos_q_p, "sin_q": sin_q_p, "cos_k": cos_k, "sin_k": sin_k,
            "masks": masks, "ones": ones, "bias": bias_p,
        })
    return in_maps


def kernel(**inputs):
    global LAST_RESULT
    from concourse.bass_utils import run_bass_kernel_spmd

    if "nc" not in _CACHE:
        _CACHE["nc"] = _build_bass()
    nc = _CACHE["nc"]

    in_maps = _host_pack(inputs)
    kwargs = {}
    if os.environ.get("KERNEL_TRACE"):
        try:
            sys.path.insert(0, os.path.dirname(os.path.abspath(__file__)))
            import axon_shim
            axon_shim.install()
        except Exception:
            pass
        kwargs["trace"] = True
    res = run_bass_kernel_spmd(nc, in_maps, core_ids=list(range(8)), **kwargs)
    LAST_RESULT = res

    out = np.empty((B, S, D_MODEL), dtype=np.float32)
    for b in range(B):
        acc = res.results[4 * b]["outT"].copy()
        for g in range(1, 4):
            acc += res.results[4 * b + g]["outT"]
        out[b] = acc.T
    return out


# revision 4
# speedup vs baseline: 1.1480x; 1.1480x over previous
"""MLA (Multi-Head Latent Attention) Bass kernel for 8 Trainium2 NeuronCores.

Sharding: 8 cores = 2 (batch) x 4 (head groups). Core c -> batch c//4,
group g=c%4 owning heads {2g, 2g+1, 2g+8, 2g+9} (paired h/h+8 so the
rotate-half RoPE over d_model=2048 stays core-local).

All activations flow on-device in transposed [feature, token] layout so no
on-chip transposes are needed (the host pre-transposes x). Attention scores
are computed in [k, q] layout; the softmax denominator is computed with an
all-ones matmul on the PE (scores are bounded, so no max subtraction), exp
runs on the scalar engine straight out of PSUM, and 1/denom is folded into
the attention-output scaling. Matmuls use fp32r (full PE rate at N=512).

Each core computes a partial out^T = (attn_out_g @ Wout[rows_g]).T for its
4 heads; the host sums the 4 partials per batch and transposes. bout is
added on-device by the g==0 cores only.
"""
import os
import sys

if "/opt/trn_rl_repo" not in sys.path:
    sys.path.insert(0, "/opt/trn_rl_repo")

import numpy as np

D_MODEL = 2048
Q_LAT = 1536
KV_LAT = 512
NUM_HEADS = 16
HD = 128
B, S = 2, 2048
SCALE = 1.0 / np.sqrt(2.0 * HD)  # 1/16

QT = 512          # query tile width (matmul free dim)
NQT = S // QT     # 4
NC_DM = D_MODEL // 128   # 16 chunks of the model dim
NC_QL = Q_LAT // 128     # 12
NC_KV = KV_LAT // 128    # 4
NKC = S // 128           # 16 key chunks

_CACHE = {}
LAST_RESULT = None


def _strip_cols(g):
    """Global column ranges (width 128) of the 4 local head strips, in local
    order [2g, 2g+1, 2g+8, 2g+9]."""
    return [256 * g, 256 * g + 128, 1024 + 256 * g, 1024 + 256 * g + 128]


def _build_bass():
    import concourse.bass as bass
    from concourse import bacc, mybir
    from concourse.tile import TileContext

    f32 = mybir.dt.float32
    f32r = mybir.dt.float32r
    AF = mybir.ActivationFunctionType

    nc = bacc.Bacc("TRN2", target_bir_lowering=False, debug=False)

    def inp(name, shape, dt=None):
        return nc.dram_tensor(name, list(shape), dt or f32r, kind="ExternalInput")

    xqT = inp("xqT", (D_MODEL, S))
    xkT = inp("xkT", (D_MODEL, S))
    wq_down = inp("wq_down", (NC_QL, 128, NC_DM * 128))     # [s][p][c*128+f]
    wkv_down = inp("wkv_down", (NC_KV, 128, NC_DM * 128))
    wk_rope = inp("wk_rope", (128, NC_DM * 128))            # [p][c*128+f]
    wq_up = inp("wq_up", (4, 128, NC_QL * 128))             # [strip][p][c*128+f]
    wq_rope = inp("wq_rope", (4, 128, NC_QL * 128))
    wk_up = inp("wk_up", (4, 128, NC_KV * 128))
    wv_up = inp("wv_up", (128, NC_KV * 512))                # [p][c*512+f]
    wout = inp("wout", (128, 64 * 128))                     # [p][(m*4+h)*128+f]
    cos_q = inp("cos_q", (2, 128, S), f32)                       # [block j][d][q]
    sin_q = inp("sin_q", (2, 128, S), f32)
    cos_k = inp("cos_k", (64, S), f32)
    sin_k = inp("sin_k", (64, S), f32)
    masks = inp("masks", (128, 4 * QT))                     # [kl][(o*QT)+ql]
    ones = inp("ones", (128, 128))
    bias = inp("bias", (128, NC_DM), f32)                        # [p][m]

    outT = nc.dram_tensor("outT", [D_MODEL, S], f32, kind="ExternalOutput")

    # DRAM scratch for inter-phase spills
    latq_d = nc.dram_tensor("latq_d", [NC_QL, 128, S], f32r, kind="Internal")
    qnew_d = nc.dram_tensor("qnew_d", [8, 128, S], f32r, kind="Internal")
    kproj_d = nc.dram_tensor("kproj_d", [4, 128, S], f32r, kind="Internal")
    krope_d = nc.dram_tensor("krope_d", [128, S], f32r, kind="Internal")
    v_d = nc.dram_tensor("v_d", [NKC, 128, 512], f32r, kind="Internal")

    def r(ap):
        return ap

    xqT_v = xqT.ap().rearrange("(c p) q -> p c q", p=128)   # [128, 16, 2048]
    xkT_v = xkT.ap().rearrange("(c p) q -> p c q", p=128)

    with TileContext(nc) as tc:
        # ------------- Phase 1: K/V build (latkv, k_proj, V, k_rope) ----
        with tc.tile_pool(name="p1w", bufs=1) as p1w, \
             tc.tile_pool(name="p1x", bufs=2) as p1x, \
             tc.tile_pool(name="p1l", bufs=1) as p1l, \
             tc.tile_pool(name="p1t", bufs=2) as p1t, \
             tc.tile_pool(name="p1ps", bufs=2, space="PSUM") as p1ps:
            wkv_sb = p1w.tile([128, NC_KV * NC_DM * 128], f32r)
            for s in range(NC_KV):
                nc.sync.dma_start(
                    out=wkv_sb[:, s * NC_DM * 128:(s + 1) * NC_DM * 128],
                    in_=wkv_down.ap()[s])
            wkr_sb = p1w.tile([128, NC_DM * 128], f32r)
            nc.sync.dma_start(out=wkr_sb, in_=wk_rope.ap())
            wku_sb = p1w.tile([128, 4 * NC_KV * 128], f32r)
            for s in range(4):
                nc.sync.dma_start(
                    out=wku_sb[:, s * NC_KV * 128:(s + 1) * NC_KV * 128],
                    in_=wk_up.ap()[s])
            wvu_sb = p1w.tile([128, NC_KV * 512], f32r)
            nc.sync.dma_start(out=wvu_sb, in_=wv_up.ap())
            cosk_sb = p1w.tile([64, S], f32)
            sink_sb = p1w.tile([64, S], f32)
            nc.sync.dma_start(out=cosk_sb, in_=cos_k.ap())
            nc.sync.dma_start(out=sink_sb, in_=sin_k.ap())

            for kh in range(2):  # k halves of 1024
                k0 = kh * 1024
                latkv = p1l.tile([128, NC_KV, 1024], f32r, tag="latkv")
                krraw = p1l.tile([128, 1024], f32, tag="krraw")
                for kt in range(2):  # two 512-tiles within the half
                    kk = k0 + kt * QT
                    xk_t = p1x.tile([128, NC_DM, QT], f32r, tag="xk")
                    nc.sync.dma_start(out=xk_t, in_=xkT_v[:, :, kk:kk + QT])
                    for s in range(NC_KV):
                        ps = p1ps.tile([128, QT], f32, tag="ps")
                        for c in range(NC_DM):
                            nc.tensor.matmul(
                                ps, r(wkv_sb[:, (s * NC_DM + c) * 128:(s * NC_DM + c + 1) * 128]),
                                r(xk_t[:, c, :]), start=(c == 0), stop=(c == NC_DM - 1))
                        nc.scalar.copy(out=latkv[:, s, kt * QT:(kt + 1) * QT], in_=ps)
                    ps = p1ps.tile([128, QT], f32, tag="ps")
                    for c in range(NC_DM):
                        nc.tensor.matmul(
                            ps, r(wkr_sb[:, c * 128:(c + 1) * 128]),
                            r(xk_t[:, c, :]), start=(c == 0), stop=(c == NC_DM - 1))
                    nc.scalar.copy(out=krraw[:, kt * QT:(kt + 1) * QT], in_=ps)

                # k_rope combine for this half
                krb = p1t.tile([64, 1024], f32, tag="krb")
                nc.sync.dma_start(out=krb, in_=krraw[64:128, :])
                ck = cosk_sb[:, k0:k0 + 1024]
                sk = sink_sb[:, k0:k0 + 1024]
                t1 = p1t.tile([64, 1024], f32, tag="krt1")
                t2 = p1t.tile([64, 1024], f32, tag="krt2")
                otop = p1t.tile([64, 1024], f32r, tag="krot")
                obot = p1t.tile([64, 1024], f32r, tag="krob")
                nc.vector.tensor_mul(t1, krraw[0:64, :], ck)
                nc.vector.tensor_mul(t2, krb, sk)
                nc.vector.tensor_sub(otop, t1, t2)
                nc.sync.dma_start(out=krope_d.ap()[0:64, k0:k0 + 1024], in_=otop)
                nc.vector.tensor_mul(t1, krb, ck)
                nc.vector.tensor_mul(t2, krraw[0:64, :], sk)
                nc.vector.tensor_add(obot, t1, t2)
                nc.sync.dma_start(out=krope_d.ap()[64:128, k0:k0 + 1024], in_=obot)

                # k_projT strips for this half
                for s in range(4):
                    for kt in range(2):
                        ps = p1ps.tile([128, QT], f32, tag="ps")
                        for c in range(NC_KV):
                            nc.tensor.matmul(
                                ps, r(wku_sb[:, (s * NC_KV + c) * 128:(s * NC_KV + c + 1) * 128]),
                                r(latkv[:, c, kt * QT:(kt + 1) * QT]),
                                start=(c == 0), stop=(c == NC_KV - 1))
                        cp = p1t.tile([128, QT], f32r, tag="kpcp")
                        nc.scalar.copy(out=cp, in_=ps)
                        nc.sync.dma_start(
                            out=kproj_d.ap()[s][:, k0 + kt * QT:k0 + (kt + 1) * QT],
                            in_=cp)
                # V natural for this half
                for kc in range(8):  # 128-chunks within the half
                    ps = p1ps.tile([128, 512], f32, tag="ps")
                    for c in range(NC_KV):
                        nc.tensor.matmul(
                            ps, r(latkv[:, c, kc * 128:(kc + 1) * 128]),
                            r(wvu_sb[:, c * 512:(c + 1) * 512]),
                            start=(c == 0), stop=(c == NC_KV - 1))
                    cp = p1t.tile([128, 512], f32r, tag="vcp")
                    nc.scalar.copy(out=cp, in_=ps)
                    nc.sync.dma_start(out=v_d.ap()[kh * 8 + kc], in_=cp)

        # ---------------- Phase 2: latqT = (xq @ Wq_down)^T -------------
        with tc.tile_pool(name="p2w", bufs=1) as p2w, \
             tc.tile_pool(name="p2x", bufs=2) as p2x, \
             tc.tile_pool(name="p2c", bufs=3) as p2c, \
             tc.tile_pool(name="p2ps", bufs=2, space="PSUM") as p2ps:
            wqd_sb = p2w.tile([128, NC_QL * NC_DM * 128], f32r)  # 96KB/part
            for s in range(NC_QL):
                nc.sync.dma_start(
                    out=wqd_sb[:, s * NC_DM * 128:(s + 1) * NC_DM * 128],
                    in_=wq_down.ap()[s])
            for qt in range(NQT):
                xq_t = p2x.tile([128, NC_DM, QT], f32r, tag="xq")
                nc.sync.dma_start(out=xq_t, in_=xqT_v[:, :, qt * QT:(qt + 1) * QT])
                for s in range(NC_QL):
                    ps = p2ps.tile([128, QT], f32, tag="ps")
                    for c in range(NC_DM):
                        nc.tensor.matmul(
                            ps, r(wqd_sb[:, (s * NC_DM + c) * 128:(s * NC_DM + c + 1) * 128]),
                            r(xq_t[:, c, :]), start=(c == 0), stop=(c == NC_DM - 1))
                    cp = p2c.tile([128, QT], f32r, tag="cp")
                    nc.scalar.copy(out=cp, in_=ps)
                    nc.sync.dma_start(
                        out=latq_d.ap()[s][:, qt * QT:(qt + 1) * QT], in_=cp)

        # ------------- Phase 3: q_projT + ropeT per head strip ----------
        with tc.tile_pool(name="p3w", bufs=1) as p3w, \
             tc.tile_pool(name="p3l", bufs=2) as p3l, \
             tc.tile_pool(name="p3t", bufs=2) as p3t, \
             tc.tile_pool(name="p3ps", bufs=2, space="PSUM") as p3ps:
            wqu_sb = p3w.tile([128, 4 * NC_QL * 128], f32r)
            wqr_sb = p3w.tile([128, 4 * NC_QL * 128], f32r)
            cosq_sb = p3w.tile([128, 2, S], f32)
            sinq_sb = p3w.tile([128, 2, S], f32)
            for s in range(4):
                nc.sync.dma_start(
                    out=wqu_sb[:, s * NC_QL * 128:(s + 1) * NC_QL * 128],
                    in_=wq_up.ap()[s])
                nc.sync.dma_start(
                    out=wqr_sb[:, s * NC_QL * 128:(s + 1) * NC_QL * 128],
                    in_=wq_rope.ap()[s])
            for j in range(2):
                nc.sync.dma_start(out=cosq_sb[:, j, :], in_=cos_q.ap()[j])
                nc.sync.dma_start(out=sinq_sb[:, j, :], in_=sin_q.ap()[j])

            for qt in range(NQT):
                q0 = qt * QT
                lat_t = p3l.tile([128, NC_QL, QT], f32r, tag="lat")
                nc.sync.dma_start(
                    out=lat_t,
                    in_=latq_d.ap().rearrange("s p q -> p s q")[:, :, q0:q0 + QT])
                raw = []
                for s in range(4):
                    # q_proj strip
                    ps = p3ps.tile([128, QT], f32, tag="ps")
                    for c in range(NC_QL):
                        nc.tensor.matmul(
                            ps, r(wqu_sb[:, (s * NC_QL + c) * 128:(s * NC_QL + c + 1) * 128]),
                            r(lat_t[:, c, :]), start=(c == 0), stop=(c == NC_QL - 1))
                    cp = p3t.tile([128, QT], f32r, tag=f"qp{s}")
                    nc.scalar.copy(out=cp, in_=ps)
                    nc.sync.dma_start(out=qnew_d.ap()[2 * s][:, q0:q0 + QT], in_=cp)
                    # q_rope raw strip
                    ps2 = p3ps.tile([128, QT], f32, tag="ps")
                    for c in range(NC_QL):
                        nc.tensor.matmul(
                            ps2, r(wqr_sb[:, (s * NC_QL + c) * 128:(s * NC_QL + c + 1) * 128]),
                            r(lat_t[:, c, :]), start=(c == 0), stop=(c == NC_QL - 1))
                    rw = p3t.tile([128, QT], f32, tag=f"raw{s}")
                    nc.scalar.copy(out=rw, in_=ps2)
                    raw.append(rw)
                # rope combine: strips 0,1 = block A (j=0,1); 2,3 = block B
                for j in range(2):
                    a, b = raw[j], raw[2 + j]
                    cj = cosq_sb[:, j, q0:q0 + QT]
                    sj = sinq_sb[:, j, q0:q0 + QT]
                    t1 = p3t.tile([128, QT], f32, tag=f"t1{j}")
                    t2 = p3t.tile([128, QT], f32, tag=f"t2{j}")
                    outa = p3t.tile([128, QT], f32r, tag=f"oa{j}")
                    outb = p3t.tile([128, QT], f32r, tag=f"ob{j}")
                    nc.vector.tensor_mul(t1, a, cj)
                    nc.vector.tensor_mul(t2, b, sj)
                    nc.vector.tensor_sub(outa, t1, t2)
                    nc.sync.dma_start(out=qnew_d.ap()[2 * j + 1][:, q0:q0 + QT], in_=outa)
                    nc.vector.tensor_mul(t1, b, cj)
                    nc.vector.tensor_mul(t2, a, sj)
                    nc.vector.tensor_add(outb, t1, t2)
                    nc.sync.dma_start(out=qnew_d.ap()[2 * (2 + j) + 1][:, q0:q0 + QT], in_=outb)

        # ------------- Phase 4: attention + output projection -----------
        with tc.tile_pool(name="p4kv", bufs=1) as p4kv, \
             tc.tile_pool(name="p4w", bufs=1) as p4w, \
             tc.tile_pool(name="p4q", bufs=2) as p4q, \
             tc.tile_pool(name="p4e", bufs=4) as p4e, \
             tc.tile_pool(name="p4a", bufs=2) as p4a, \
             tc.tile_pool(name="p4o", bufs=2) as p4o, \
             tc.tile_pool(name="p4ps", bufs=2, space="PSUM") as p4ps, \
             tc.tile_pool(name="p4pd", bufs=2, space="PSUM") as p4pd, \
             tc.tile_pool(name="p4pv", bufs=2, space="PSUM") as p4pv, \
             tc.tile_pool(name="p4po", bufs=2, space="PSUM") as p4po:
            kproj_sb = p4kv.tile([128, 4, S], f32r)
            nc.sync.dma_start(out=kproj_sb, in_=kproj_d.ap().rearrange("s p k -> p s k"))
            krope_sb = p4kv.tile([128, S], f32r)
            nc.sync.dma_start(out=krope_sb, in_=krope_d.ap())
            v_sb = p4kv.tile([128, NKC, 512], f32r)
            nc.sync.dma_start(out=v_sb, in_=v_d.ap().rearrange("c p f -> p c f"))
            wout_sb = p4w.tile([128, 64 * 128], f32r)
            nc.sync.dma_start(out=wout_sb, in_=wout.ap())
            masks_sb = p4w.tile([128, 4 * QT], f32r)
            nc.sync.dma_start(out=masks_sb, in_=masks.ap())
            ones_sb = p4w.tile([128, 128], f32r)
            nc.sync.dma_start(out=ones_sb, in_=ones.ap())
            bias_sb = p4w.tile([128, NC_DM], f32)
            nc.sync.dma_start(out=bias_sb, in_=bias.ap())

            for qt in range(NQT):
                q0 = qt * QT
                K = (q0 + QT) // 128  # causal: chunks 0..K-1
                qn = p4q.tile([128, 8, QT], f32r, tag="qn")
                nc.sync.dma_start(
                    out=qn, in_=qnew_d.ap().rearrange("s p q -> p s q")[:, :, q0:q0 + QT])
                attn = p4a.tile([128, 4, QT], f32r, tag="attn")
                for h in range(4):
                    psd = p4pd.tile([128, QT], f32, tag="psd")
                    psv = p4pv.tile([128, QT], f32, tag="psv")
                    for kc in range(K):
                        pss = p4ps.tile([128, QT], f32, tag="pss")
                        nc.tensor.matmul(
                            pss, r(kproj_sb[:, h, kc * 128:(kc + 1) * 128]),
                            r(qn[:, 2 * h, :]), start=True, stop=False)
                        nc.tensor.matmul(
                            pss, r(krope_sb[:, kc * 128:(kc + 1) * 128]),
                            r(qn[:, 2 * h + 1, :]), start=False, stop=True)
                        ex = p4e.tile([128, QT], f32r, tag="ex")
                        nc.scalar.activation(out=ex, in_=pss, func=AF.Exp, scale=float(SCALE))
                        o = kc - q0 // 128
                        if o >= 0:  # diagonal chunk: apply causal mask
                            nc.vector.tensor_mul(ex, ex, masks_sb[:, o * QT:(o + 1) * QT])
                        nc.tensor.matmul(
                            psd, r(ones_sb), r(ex),
                            start=(kc == 0), stop=(kc == K - 1), skip_group_check=True)
                        nc.tensor.matmul(
                            psv, r(v_sb[:, kc, h * 128:(h + 1) * 128]), r(ex),
                            start=(kc == 0), stop=(kc == K - 1), skip_group_check=True)
                    rec = p4e.tile([128, QT], f32, tag="rec")
                    nc.vector.reciprocal_approx_fast(out=rec, in_=psd)
                    nc.vector.tensor_mul(attn[:, h, :], psv, rec)
                # output projection for this q tile
                for m in range(NC_DM):
                    pso = p4po.tile([128, QT], f32, tag="pso")
                    for h in range(4):
                        nc.tensor.matmul(
                            pso, r(wout_sb[:, (m * 4 + h) * 128:(m * 4 + h + 1) * 128]),
                            r(attn[:, h, :]), start=(h == 0), stop=(h == 3))
                    oc = p4o.tile([128, QT], f32, tag="oc")
                    nc.scalar.activation(
                        out=oc, in_=pso, func=AF.Identity,
                        bias=bias_sb[:, m:m + 1], scale=1.0)
                    nc.sync.dma_start(
                        out=outT.ap()[m * 128:(m + 1) * 128, q0:q0 + QT], in_=oc)

    nc.finalize()
    return nc


def _host_pack(inputs):
    """Build the 8 per-core input maps from the full inputs."""
    xq = np.ascontiguousarray(inputs["inputs_q"], dtype=np.float32)
    xk = np.ascontiguousarray(inputs["inputs_k"], dtype=np.float32)
    Wq_down = np.asarray(inputs["Wq_down"], dtype=np.float32)
    Wkv_down = np.asarray(inputs["Wkv_down"], dtype=np.float32)
    Wq_up = np.asarray(inputs["Wq_up"], dtype=np.float32)
    Wk_up = np.asarray(inputs["Wk_up"], dtype=np.float32)
    Wv_up = np.asarray(inputs["Wv_up"], dtype=np.float32)
    Wq_rope = np.asarray(inputs["Wq_rope"], dtype=np.float32)
    Wk_rope = np.asarray(inputs["Wk_rope"], dtype=np.float32)
    Wout = np.asarray(inputs["Wout"], dtype=np.float32)
    bout = np.asarray(inputs["bout"], dtype=np.float32)

    def pack_lhs(W, n_strips, strip_starts, nchunks):
        # -> [n_strips, 128, nchunks*128]: [s][p][c*128+f]
        out = np.empty((n_strips, 128, nchunks * 128), dtype=np.float32)
        for s in range(n_strips):
            blk = W[:, strip_starts[s]:strip_starts[s] + 128]  # [nchunks*128, 128]
            out[s] = blk.reshape(nchunks, 128, 128).transpose(1, 0, 2).reshape(128, -1)
        return out

    xqT = [np.ascontiguousarray(xq[b].T) for b in range(B)]
    xkT = [np.ascontiguousarray(xk[b].T) for b in range(B)]

    wq_down_p = pack_lhs(Wq_down, NC_QL, [128 * s for s in range(NC_QL)], NC_DM)
    wkv_down_p = pack_lhs(Wkv_down, NC_KV, [128 * s for s in range(NC_KV)], NC_DM)
    wk_rope_p = pack_lhs(Wk_rope, 1, [0], NC_DM)[0]

    # rope tables
    iq = np.arange(1024, dtype=np.float64)
    inv_q = 1.0 / (10000.0 ** (iq * 2.0 / D_MODEL))
    pos = np.arange(S, dtype=np.float64)
    ang_q = pos[:, None] * inv_q[None, :]          # [S, 1024]
    ik = np.arange(64, dtype=np.float64)
    inv_k = 1.0 / (10000.0 ** (ik * 2.0 / HD))
    ang_k = pos[:, None] * inv_k[None, :]          # [S, 64]
    cos_k = np.ascontiguousarray(np.cos(ang_k).T.astype(np.float32))  # [64, S]
    sin_k = np.ascontiguousarray(np.sin(ang_k).T.astype(np.float32))

    # causal diag masks [128, 4*QT]
    kl = np.arange(128)[:, None]
    ql = np.arange(QT)[None, :]
    masks = np.concatenate(
        [(kl + 128 * o <= ql).astype(np.float32) for o in range(4)], axis=1)
    masks = np.ascontiguousarray(masks)
    ones = np.ones((128, 128), dtype=np.float32)

    in_maps = []
    for c in range(8):
        b, g = divmod(c, 4)
        cols = _strip_cols(g)
        wq_up_p = pack_lhs(Wq_up, 4, cols, NC_QL)
        wq_rope_p = pack_lhs(Wq_rope, 4, cols, NC_QL)
        wk_up_p = pack_lhs(Wk_up, 4, cols, NC_KV)
        # wv_up: [128, nc_kv*512]; cols4 concatenated in local order
        cols4 = np.concatenate([np.arange(cs, cs + 128) for cs in cols])
        Wv_g = Wv_up[:, cols4]                      # [512, 512]
        wv_up_p = Wv_g.reshape(NC_KV, 128, 512).transpose(1, 0, 2).reshape(128, -1)
        # wout: rows for local heads; [128, 64*128] = [p][(m*4+h)*128+f]
        Wout_g = Wout[cols4, :].reshape(4, 128, NC_DM, 128)   # [h][p][m][f]
        wout_p = np.ascontiguousarray(
            Wout_g.transpose(1, 2, 0, 3).reshape(128, -1))    # [p][m,h,f]
        # cos/sin q for blocks j=0,1: global cols 256g+128j+d (<1024)
        cos_q_p = np.empty((2, 128, S), dtype=np.float32)
        sin_q_p = np.empty((2, 128, S), dtype=np.float32)
        for j in range(2):
            idx = 256 * g + 128 * j + np.arange(128)
            cos_q_p[j] = np.cos(ang_q[:, idx]).T
            sin_q_p[j] = np.sin(ang_q[:, idx]).T
        bias_p = (bout if g == 0 else np.zeros_like(bout)).reshape(NC_DM, 128)
        bias_p = np.ascontiguousarray(bias_p.T)     # [128, m]

        in_maps.append({
            "xqT": xqT[b], "xkT": xkT[b],
            "wq_down": wq_down_p, "wkv_down": wkv_down_p, "wk_rope": wk_rope_p,
            "wq_up": wq_up_p, "wq_rope": wq_rope_p, "wk_up": wk_up_p,
            "wv_up": np.ascontiguousarray(wv_up_p), "wout": wout_p,
            "cos_q": cos_q_p, "sin_q": sin_q_p, "cos_k": cos_k, "sin_k": sin_k,
            "masks": masks, "ones": ones, "bias": bias_p,
        })
    return in_maps


def kernel(**inputs):
    global LAST_RESULT
    from concourse.bass_utils import run_bass_kernel_spmd

    if "nc" not in _CACHE:
        _CACHE["nc"] = _build_bass()
    nc = _CACHE["nc"]

    in_maps = _host_pack(inputs)
    kwargs = {}
    if os.environ.get("KERNEL_TRACE"):
        try:
            sys.path.insert(0, os.path.dirname(os.path.abspath(__file__)))
            import axon_shim
            axon_shim.install()
        except Exception:
            pass
        kwargs["trace"] = True
    res = run_bass_kernel_spmd(nc, in_maps, core_ids=list(range(8)), **kwargs)
    LAST_RESULT = res

    out = np.empty((B, S, D_MODEL), dtype=np.float32)
    for b in range(B):
        acc = res.results[4 * b]["outT"].copy()
        for g in range(1, 4):
            acc += res.results[4 * b + g]["outT"]
        out[b] = acc.T
    return out


# revision 5
# speedup vs baseline: 1.4712x; 1.2816x over previous
"""MLA (Multi-Head Latent Attention) Bass kernel for 8 Trainium2 NeuronCores.

Sharding: 8 cores = 2 (batch) x 4 (head groups). Core c -> batch c//4,
group g=c%4 owning heads {2g, 2g+1, 2g+8, 2g+9} (paired h/h+8 so the
rotate-half RoPE over d_model=2048 stays core-local).

All activations flow on-device in transposed [feature, token] layout so no
on-chip transposes are needed (the host pre-transposes x). Attention scores
are computed in [k, q] layout; the softmax denominator is computed with an
all-ones matmul on the PE (scores are bounded, so no max subtraction), exp
runs on the scalar engine straight out of PSUM, and 1/denom is folded into
the attention-output scaling.

Matmuls run in bf16 (fp32 PSUM accumulation). The core folds its slice of
Wq_down @ Wq_up (and @ Wq_rope) on-device first — 6.4 GFLOP of folding
replaces 19.3 GFLOP of replicated latent-Q work per core. K/V and q_new
stay resident in SBUF; only the folded weights round-trip through DRAM.

Each core computes a partial out^T = (attn_out_g @ Wout[rows_g]).T for its
4 heads; the host sums the 4 partials per batch and transposes. bout is
added on-device by the g==0 cores only.
"""
import os
import sys

if "/opt/trn_rl_repo" not in sys.path:
    sys.path.insert(0, "/opt/trn_rl_repo")

import numpy as np

D_MODEL = 2048
Q_LAT = 1536
KV_LAT = 512
NUM_HEADS = 16
HD = 128
B, S = 2, 2048
SCALE = 1.0 / np.sqrt(2.0 * HD)  # 1/16

QT = 512          # query tile width (matmul free dim)
NQT = S // QT     # 4
NC_DM = D_MODEL // 128   # 16 chunks of the model dim
NC_QL = Q_LAT // 128     # 12
NC_KV = KV_LAT // 128    # 4
NKC = S // 128           # 16 key chunks

_CACHE = {}
LAST_RESULT = None


def _strip_cols(g):
    """Global column starts (width 128) of the 4 local head strips, in local
    order [2g, 2g+1, 2g+8, 2g+9]."""
    return [256 * g, 256 * g + 128, 1024 + 256 * g, 1024 + 256 * g + 128]


def _build_bass():
    from concourse import bacc, mybir
    from concourse.tile import TileContext

    f32 = mybir.dt.float32
    bf16 = mybir.dt.bfloat16
    AF = mybir.ActivationFunctionType

    nc = bacc.Bacc("TRN2", target_bir_lowering=False, debug=False)

    def inp(name, shape, dt=bf16):
        return nc.dram_tensor(name, list(shape), dt, kind="ExternalInput")

    xqT = inp("xqT", (D_MODEL, S))
    xkT = inp("xkT", (D_MODEL, S))
    # Wq_down^T tiles for the fold: [lat-chunk l][p=lat][c*128+f] (f over dm)
    wq_downT = inp("wq_downT", (NC_QL, 128, NC_DM * 128))
    wkv_down = inp("wkv_down", (NC_KV, 128, NC_DM * 128))  # [s][p=dm][c*128+f]
    wk_rope = inp("wk_rope", (128, NC_DM * 128))           # [p=dm][c*128+f]
    # up-proj slices for the fold: [p=lat][l(lat-chunk)*512 + f(4 strips x 128)]
    wq_up = inp("wq_up", (128, NC_QL * 512))
    wq_rope = inp("wq_rope", (128, NC_QL * 512))
    wk_up = inp("wk_up", (4, 128, NC_KV * 128))            # [strip][p=lat][c*128+f]
    wv_up = inp("wv_up", (128, NC_KV * 512))               # [p=lat][c*512+f]
    wout = inp("wout", (128, 64 * 128))                    # [p][(m*4+h)*128+f]
    cos_q = inp("cos_q", (2, 128, S), f32)                 # [block j][d][q]
    sin_q = inp("sin_q", (2, 128, S), f32)
    cos_k = inp("cos_k", (64, S), f32)
    sin_k = inp("sin_k", (64, S), f32)
    masks = inp("masks", (128, 4 * QT))                    # [kl][(o*QT)+ql]
    ones = inp("ones", (128, 128))
    bias = inp("bias", (128, NC_DM), f32)                  # [p][m]

    outT = nc.dram_tensor("outT", [D_MODEL, S], f32, kind="ExternalOutput")

    # folded Weff spill: [kind][dm-chunk c][p=dm][f=4 strips x 128]
    weff_d = nc.dram_tensor("weff_d", [2, NC_DM, 128, 512], bf16, kind="Internal")

    xqT_v = xqT.ap().rearrange("(c p) q -> p c q", p=128)  # [128, 16, 2048]
    xkT_v = xkT.ap().rearrange("(c p) q -> p c q", p=128)

    with TileContext(nc) as tc:
        with tc.tile_pool(name="kvres", bufs=1) as kvres, \
             tc.tile_pool(name="qnres", bufs=1) as qnres:
            # resident outputs of phase A1 / A2 (consumed in phase B)
            kproj_sb = kvres.tile([128, 4, S], bf16)
            krope_sb = kvres.tile([128, S], bf16)
            v_sb = kvres.tile([128, NKC, 512], bf16)
            qn_sb = qnres.tile([128, 8, S], bf16)  # [2*strip + (0=proj,1=rope)]

            # ---------------- Phase F: fold Weff = Wq_down @ Wq_up|rope ----
            with tc.tile_pool(name="pfw", bufs=1) as pfw, \
                 tc.tile_pool(name="pfc", bufs=3) as pfc, \
                 tc.tile_pool(name="pfps", bufs=2, space="PSUM") as pfps:
                wqdT_sb = pfw.tile([128, NC_QL * NC_DM * 128], bf16)
                for l in range(NC_QL):
                    nc.sync.dma_start(
                        out=wqdT_sb[:, l * NC_DM * 128:(l + 1) * NC_DM * 128],
                        in_=wq_downT.ap()[l])
                wqu_sb = pfw.tile([128, NC_QL * 512], bf16)
                wqr_sb = pfw.tile([128, NC_QL * 512], bf16)
                nc.sync.dma_start(out=wqu_sb, in_=wq_up.ap())
                nc.sync.dma_start(out=wqr_sb, in_=wq_rope.ap())
                for kind, wup in ((0, wqu_sb), (1, wqr_sb)):
                    for c in range(NC_DM):
                        ps = pfps.tile([128, 512], f32, tag="ps")
                        for l in range(NC_QL):
                            nc.tensor.matmul(
                                ps,
                                wqdT_sb[:, (l * NC_DM + c) * 128:(l * NC_DM + c + 1) * 128],
                                wup[:, l * 512:(l + 1) * 512],
                                start=(l == 0), stop=(l == NC_QL - 1))
                        cp = pfc.tile([128, 512], bf16, tag="cp")
                        nc.scalar.copy(out=cp, in_=ps)
                        nc.sync.dma_start(out=weff_d.ap()[kind][c], in_=cp)

            # ------------- Phase A1: K/V build (latkv, k_proj, V, k_rope) --
            with tc.tile_pool(name="a1w", bufs=1) as a1w, \
                 tc.tile_pool(name="a1x", bufs=2) as a1x, \
                 tc.tile_pool(name="a1t", bufs=2) as a1t, \
                 tc.tile_pool(name="a1ps", bufs=2, space="PSUM") as a1ps:
                wkv_sb = a1w.tile([128, NC_KV * NC_DM * 128], bf16)
                for s in range(NC_KV):
                    nc.sync.dma_start(
                        out=wkv_sb[:, s * NC_DM * 128:(s + 1) * NC_DM * 128],
                        in_=wkv_down.ap()[s])
                wkr_sb = a1w.tile([128, NC_DM * 128], bf16)
                nc.sync.dma_start(out=wkr_sb, in_=wk_rope.ap())
                wku_sb = a1w.tile([128, 4 * NC_KV * 128], bf16)
                for s in range(4):
                    nc.sync.dma_start(
                        out=wku_sb[:, s * NC_KV * 128:(s + 1) * NC_KV * 128],
                        in_=wk_up.ap()[s])
                wvu_sb = a1w.tile([128, NC_KV * 512], bf16)
                nc.sync.dma_start(out=wvu_sb, in_=wv_up.ap())
                cosk_sb = a1w.tile([64, S], f32)
                sink_sb = a1w.tile([64, S], f32)
                nc.sync.dma_start(out=cosk_sb, in_=cos_k.ap())
                nc.sync.dma_start(out=sink_sb, in_=sin_k.ap())

                for kt in range(4):  # k tiles of 512
                    k0 = kt * QT
                    xk_t = a1x.tile([128, NC_DM, QT], bf16, tag="xk")
                    nc.sync.dma_start(out=xk_t, in_=xkT_v[:, :, k0:k0 + QT])
                    latkv = a1t.tile([128, NC_KV, QT], bf16, tag="latkv")
                    for s in range(NC_KV):
                        ps = a1ps.tile([128, QT], f32, tag="ps")
                        for c in range(NC_DM):
                            nc.tensor.matmul(
                                ps, wkv_sb[:, (s * NC_DM + c) * 128:(s * NC_DM + c + 1) * 128],
                                xk_t[:, c, :], start=(c == 0), stop=(c == NC_DM - 1))
                        nc.scalar.copy(out=latkv[:, s, :], in_=ps)
                    # k_rope raw
                    ps = a1ps.tile([128, QT], f32, tag="ps")
                    for c in range(NC_DM):
                        nc.tensor.matmul(
                            ps, wkr_sb[:, c * 128:(c + 1) * 128],
                            xk_t[:, c, :], start=(c == 0), stop=(c == NC_DM - 1))
                    krraw = a1t.tile([128, QT], f32, tag="krraw")
                    nc.scalar.copy(out=krraw, in_=ps)
                    krb = a1t.tile([64, QT], f32, tag="krb")
                    nc.sync.dma_start(out=krb, in_=krraw[64:128, :])
                    ck = cosk_sb[:, k0:k0 + QT]
                    sk = sink_sb[:, k0:k0 + QT]
                    t1 = a1t.tile([64, QT], f32, tag="krt1")
                    t2 = a1t.tile([64, QT], f32, tag="krt2")
                    nc.vector.tensor_mul(t1, krraw[0:64, :], ck)
                    nc.vector.tensor_mul(t2, krb, sk)
                    nc.vector.tensor_sub(krope_sb[0:64, k0:k0 + QT], t1, t2)
                    obot = a1t.tile([64, QT], bf16, tag="krob")
                    nc.vector.tensor_mul(t1, krb, ck)
                    nc.vector.tensor_mul(t2, krraw[0:64, :], sk)
                    nc.vector.tensor_add(obot, t1, t2)
                    nc.sync.dma_start(out=krope_sb[64:128, k0:k0 + QT], in_=obot)
                    # k_projT strips
                    for s in range(4):
                        ps = a1ps.tile([128, QT], f32, tag="ps")
                        for c in range(NC_KV):
                            nc.tensor.matmul(
                                ps, wku_sb[:, (s * NC_KV + c) * 128:(s * NC_KV + c + 1) * 128],
                                latkv[:, c, :], start=(c == 0), stop=(c == NC_KV - 1))
                        nc.scalar.copy(out=kproj_sb[:, s, k0:k0 + QT], in_=ps)
                    # V natural
                    for kc in range(4):  # 128-chunks within this k tile
                        ps = a1ps.tile([128, 512], f32, tag="ps")
                        for c in range(NC_KV):
                            nc.tensor.matmul(
                                ps, latkv[:, c, kc * 128:(kc + 1) * 128],
                                wvu_sb[:, c * 512:(c + 1) * 512],
                                start=(c == 0), stop=(c == NC_KV - 1))
                        nc.scalar.copy(out=v_sb[:, kt * 4 + kc, :], in_=ps)

            # ------------- Phase A2a: q_proj strips from x and Weff --------
            with tc.tile_pool(name="a2w", bufs=1) as a2w, \
                 tc.tile_pool(name="a2x", bufs=2) as a2x, \
                 tc.tile_pool(name="a2ps", bufs=2, space="PSUM") as a2ps:
                weffa_sb = a2w.tile([128, NC_DM, 512], bf16)
                nc.sync.dma_start(out=weffa_sb, in_=weff_d.ap().rearrange(
                    "k c p f -> p k c f")[:, 0])
                for qt in range(NQT):
                    q0 = qt * QT
                    xq_t = a2x.tile([128, NC_DM, QT], bf16, tag="xq")
                    nc.sync.dma_start(out=xq_t, in_=xqT_v[:, :, q0:q0 + QT])
                    for s in range(4):
                        ps = a2ps.tile([128, QT], f32, tag="ps")
                        for c in range(NC_DM):
                            nc.tensor.matmul(
                                ps, weffa_sb[:, c, s * 128:(s + 1) * 128],
                                xq_t[:, c, :], start=(c == 0), stop=(c == NC_DM - 1))
                        nc.scalar.copy(out=qn_sb[:, 2 * s, q0:q0 + QT], in_=ps)

            # ------------- Phase A2b: q_rope strips from x and Weff --------
            with tc.tile_pool(name="a3w", bufs=1) as a3w, \
                 tc.tile_pool(name="a3x", bufs=2) as a3x, \
                 tc.tile_pool(name="a3t", bufs=2) as a3t, \
                 tc.tile_pool(name="a3ps", bufs=2, space="PSUM") as a3ps:
                weffb_sb = a3w.tile([128, NC_DM, 512], bf16)
                nc.sync.dma_start(out=weffb_sb, in_=weff_d.ap().rearrange(
                    "k c p f -> p k c f")[:, 1])
                cosq_sb = a3w.tile([128, 2, S], f32)
                sinq_sb = a3w.tile([128, 2, S], f32)
                for j in range(2):
                    nc.sync.dma_start(out=cosq_sb[:, j, :], in_=cos_q.ap()[j])
                    nc.sync.dma_start(out=sinq_sb[:, j, :], in_=sin_q.ap()[j])
                for qt in range(NQT):
                    q0 = qt * QT
                    xq_t = a3x.tile([128, NC_DM, QT], bf16, tag="xq")
                    nc.sync.dma_start(out=xq_t, in_=xqT_v[:, :, q0:q0 + QT])
                    raw = []
                    for s in range(4):
                        ps = a3ps.tile([128, QT], f32, tag="ps")
                        for c in range(NC_DM):
                            nc.tensor.matmul(
                                ps, weffb_sb[:, c, s * 128:(s + 1) * 128],
                                xq_t[:, c, :], start=(c == 0), stop=(c == NC_DM - 1))
                        rw = a3t.tile([128, QT], f32, tag=f"raw{s}")
                        nc.scalar.copy(out=rw, in_=ps)
                        raw.append(rw)
                    for j in range(2):
                        a, b = raw[j], raw[2 + j]
                        cj = cosq_sb[:, j, q0:q0 + QT]
                        sj = sinq_sb[:, j, q0:q0 + QT]
                        t1 = a3t.tile([128, QT], f32, tag=f"t1{j}")
                        t2 = a3t.tile([128, QT], f32, tag=f"t2{j}")
                        nc.vector.tensor_mul(t1, a, cj)
                        nc.vector.tensor_mul(t2, b, sj)
                        nc.vector.tensor_sub(qn_sb[:, 2 * j + 1, q0:q0 + QT], t1, t2)
                        nc.vector.tensor_mul(t1, b, cj)
                        nc.vector.tensor_mul(t2, a, sj)
                        nc.vector.tensor_add(qn_sb[:, 2 * (2 + j) + 1, q0:q0 + QT], t1, t2)

            # ------------- Phase B: attention + output projection ----------
            with tc.tile_pool(name="bw", bufs=1) as bw, \
                 tc.tile_pool(name="be", bufs=4) as be, \
                 tc.tile_pool(name="ba", bufs=2) as ba, \
                 tc.tile_pool(name="bo", bufs=2) as bo, \
                 tc.tile_pool(name="bps", bufs=2, space="PSUM") as bps, \
                 tc.tile_pool(name="bpd", bufs=2, space="PSUM") as bpd, \
                 tc.tile_pool(name="bpv", bufs=2, space="PSUM") as bpv, \
                 tc.tile_pool(name="bpo", bufs=2, space="PSUM") as bpo:
                wout_sb = bw.tile([128, 64 * 128], bf16)
                nc.sync.dma_start(out=wout_sb, in_=wout.ap())
                masks_sb = bw.tile([128, 4 * QT], bf16)
                nc.sync.dma_start(out=masks_sb, in_=masks.ap())
                ones_sb = bw.tile([128, 128], bf16)
                nc.sync.dma_start(out=ones_sb, in_=ones.ap())
                bias_sb = bw.tile([128, NC_DM], f32)
                nc.sync.dma_start(out=bias_sb, in_=bias.ap())

                for qt in range(NQT):
                    q0 = qt * QT
                    K = (q0 + QT) // 128  # causal: chunks 0..K-1
                    attn = ba.tile([128, 4, QT], bf16, tag="attn")
                    for h in range(4):
                        psd = bpd.tile([128, QT], f32, tag="psd")
                        psv = bpv.tile([128, QT], f32, tag="psv")
                        for kc in range(K):
                            pss = bps.tile([128, QT], f32, tag="pss")
                            nc.tensor.matmul(
                                pss, kproj_sb[:, h, kc * 128:(kc + 1) * 128],
                                qn_sb[:, 2 * h, q0:q0 + QT], start=True, stop=False)
                            nc.tensor.matmul(
                                pss, krope_sb[:, kc * 128:(kc + 1) * 128],
                                qn_sb[:, 2 * h + 1, q0:q0 + QT], start=False, stop=True)
                            ex = be.tile([128, QT], bf16, tag="ex")
                            nc.scalar.activation(out=ex, in_=pss, func=AF.Exp,
                                                 scale=float(SCALE))
                            o = kc - q0 // 128
                            if o >= 0:  # diagonal chunk: apply causal mask
                                nc.vector.tensor_mul(
                                    ex, ex, masks_sb[:, o * QT:(o + 1) * QT])
                            nc.tensor.matmul(
                                psd, ones_sb, ex,
                                start=(kc == 0), stop=(kc == K - 1),
                                skip_group_check=True)
                            nc.tensor.matmul(
                                psv, v_sb[:, kc, h * 128:(h + 1) * 128], ex,
                                start=(kc == 0), stop=(kc == K - 1),
                                skip_group_check=True)
                        rec = be.tile([128, QT], f32, tag="rec")
                        nc.vector.reciprocal_approx_fast(out=rec, in_=psd)
                        nc.vector.tensor_mul(attn[:, h, :], psv, rec)
                    # output projection for this q tile
                    for m in range(NC_DM):
                        pso = bpo.tile([128, QT], f32, tag="pso")
                        for h in range(4):
                            nc.tensor.matmul(
                                pso, wout_sb[:, (m * 4 + h) * 128:(m * 4 + h + 1) * 128],
                                attn[:, h, :], start=(h == 0), stop=(h == 3))
                        oc = bo.tile([128, QT], f32, tag="oc")
                        nc.scalar.activation(
                            out=oc, in_=pso, func=AF.Identity,
                            bias=bias_sb[:, m:m + 1], scale=1.0)
                        nc.sync.dma_start(
                            out=outT.ap()[m * 128:(m + 1) * 128, q0:q0 + QT], in_=oc)

    nc.finalize()
    return nc


def _host_pack(inputs):
    """Build the 8 per-core input maps from the full inputs."""
    import ml_dtypes
    bf16 = ml_dtypes.bfloat16

    xq = np.ascontiguousarray(inputs["inputs_q"], dtype=np.float32)
    xk = np.ascontiguousarray(inputs["inputs_k"], dtype=np.float32)
    Wq_down = np.asarray(inputs["Wq_down"], dtype=np.float32)
    Wkv_down = np.asarray(inputs["Wkv_down"], dtype=np.float32)
    Wq_up = np.asarray(inputs["Wq_up"], dtype=np.float32)
    Wk_up = np.asarray(inputs["Wk_up"], dtype=np.float32)
    Wv_up = np.asarray(inputs["Wv_up"], dtype=np.float32)
    Wq_rope = np.asarray(inputs["Wq_rope"], dtype=np.float32)
    Wk_rope = np.asarray(inputs["Wk_rope"], dtype=np.float32)
    Wout = np.asarray(inputs["Wout"], dtype=np.float32)
    bout = np.asarray(inputs["bout"], dtype=np.float32)

    def pack_lhs(W, n_strips, strip_starts, nchunks):
        # -> [n_strips, 128, nchunks*128]: [s][p][c*128+f]
        out = np.empty((n_strips, 128, nchunks * 128), dtype=bf16)
        for s in range(n_strips):
            blk = W[:, strip_starts[s]:strip_starts[s] + 128]  # [nchunks*128, 128]
            out[s] = blk.reshape(nchunks, 128, 128).transpose(1, 0, 2).reshape(128, -1).astype(bf16)
        return out

    xqT = [np.ascontiguousarray(xq[b].T.astype(bf16)) for b in range(B)]
    xkT = [np.ascontiguousarray(xk[b].T.astype(bf16)) for b in range(B)]

    # Wq_down^T packed for the fold (lhsT tiles [p=lat, f=dm])
    WqdT = np.ascontiguousarray(Wq_down.T)  # [Q_LAT, D_MODEL]
    wq_downT_p = np.empty((NC_QL, 128, NC_DM * 128), dtype=bf16)
    for l in range(NC_QL):
        wq_downT_p[l] = WqdT[128 * l:128 * (l + 1), :].astype(bf16)

    wkv_down_p = pack_lhs(Wkv_down, NC_KV, [128 * s for s in range(NC_KV)], NC_DM)
    wk_rope_p = pack_lhs(Wk_rope, 1, [0], NC_DM)[0]

    # rope tables (fp32)
    iq = np.arange(1024, dtype=np.float64)
    inv_q = 1.0 / (10000.0 ** (iq * 2.0 / D_MODEL))
    pos = np.arange(S, dtype=np.float64)
    ang_q = pos[:, None] * inv_q[None, :]          # [S, 1024]
    ik = np.arange(64, dtype=np.float64)
    inv_k = 1.0 / (10000.0 ** (ik * 2.0 / HD))
    ang_k = pos[:, None] * inv_k[None, :]          # [S, 64]
    cos_k = np.ascontiguousarray(np.cos(ang_k).T.astype(np.float32))  # [64, S]
    sin_k = np.ascontiguousarray(np.sin(ang_k).T.astype(np.float32))

    kl = np.arange(128)[:, None]
    ql = np.arange(QT)[None, :]
    masks = np.concatenate(
        [(kl + 128 * o <= ql).astype(np.float32) for o in range(4)], axis=1)
    masks = np.ascontiguousarray(masks.astype(bf16))
    ones = np.ones((128, 128), dtype=bf16)

    in_maps = []
    for c in range(8):
        b, g = divmod(c, 4)
        cols = _strip_cols(g)
        cols4 = np.concatenate([np.arange(cs, cs + 128) for cs in cols])

        # up-proj slices for fold rhs: [p=lat within chunk][l*512 + f]
        def pack_up(W):
            Wg = W[:, cols4]  # [Q_LAT, 512]
            return np.ascontiguousarray(
                Wg.reshape(NC_QL, 128, 512).transpose(1, 0, 2).reshape(128, -1).astype(bf16))
        wq_up_p = pack_up(Wq_up)
        wq_rope_p = pack_up(Wq_rope)
        wk_up_p = pack_lhs(Wk_up, 4, cols, NC_KV)
        Wv_g = Wv_up[:, cols4]                      # [512, 512]
        wv_up_p = np.ascontiguousarray(
            Wv_g.reshape(NC_KV, 128, 512).transpose(1, 0, 2).reshape(128, -1).astype(bf16))
        Wout_g = Wout[cols4, :].reshape(4, 128, NC_DM, 128)   # [h][p][m][f]
        wout_p = np.ascontiguousarray(
            Wout_g.transpose(1, 2, 0, 3).reshape(128, -1).astype(bf16))
        cos_q_p = np.empty((2, 128, S), dtype=np.float32)
        sin_q_p = np.empty((2, 128, S), dtype=np.float32)
        for j in range(2):
            idx = 256 * g + 128 * j + np.arange(128)
            cos_q_p[j] = np.cos(ang_q[:, idx]).T
            sin_q_p[j] = np.sin(ang_q[:, idx]).T
        bias_p = (bout if g == 0 else np.zeros_like(bout)).reshape(NC_DM, 128)
        bias_p = np.ascontiguousarray(bias_p.T)     # [128, m]

        in_maps.append({
            "xqT": xqT[b], "xkT": xkT[b],
            "wq_downT": wq_downT_p, "wkv_down": wkv_down_p, "wk_rope": wk_rope_p,
            "wq_up": wq_up_p, "wq_rope": wq_rope_p, "wk_up": wk_up_p,
            "wv_up": wv_up_p, "wout": wout_p,
            "cos_q": cos_q_p, "sin_q": sin_q_p, "cos_k": cos_k, "sin_k": sin_k,
            "masks": masks, "ones": ones, "bias": bias_p,
        })
    return in_maps


def kernel(**inputs):
    global LAST_RESULT
    from concourse.bass_utils import run_bass_kernel_spmd

    if "nc" not in _CACHE:
        _CACHE["nc"] = _build_bass()
    nc = _CACHE["nc"]

    in_maps = _host_pack(inputs)
    kwargs = {}
    if os.environ.get("KERNEL_TRACE"):
        try:
            sys.path.insert(0, os.path.dirname(os.path.abspath(__file__)))
            import axon_shim
            axon_shim.install()
        except Exception:
            pass
        kwargs["trace"] = True
    res = run_bass_kernel_spmd(nc, in_maps, core_ids=list(range(8)), **kwargs)
    LAST_RESULT = res

    out = np.empty((B, S, D_MODEL), dtype=np.float32)
    for b in range(B):
        acc = res.results[4 * b]["outT"].copy()
        for g in range(1, 4):
            acc += res.results[4 * b + g]["outT"]
        out[b] = acc.T
    return out


# revision 6
# speedup vs baseline: 1.5725x; 1.0688x over previous
"""MLA (Multi-Head Latent Attention) Bass kernel for 8 Trainium2 NeuronCores.

Sharding: 8 cores = 2 (batch) x 4 (head groups). Core c -> batch c//4,
group g=c%4 owning heads {2g, 2g+1, 2g+8, 2g+9} (paired h/h+8 so the
rotate-half RoPE over d_model=2048 stays core-local).

All activations flow on-device in transposed [feature, token] layout so no
on-chip transposes are needed (the host pre-transposes x). Attention scores
are computed in [k, q] layout; the softmax denominator is computed with an
all-ones matmul on the PE (scores are bounded, so no max subtraction), exp
runs on the scalar engine straight out of PSUM, and 1/denom is folded into
the attention-output scaling.

Matmuls run in bf16 (fp32 PSUM accumulation). The core folds its slice of
Wq_down @ Wq_up (and @ Wq_rope) on-device first — 6.4 GFLOP of folding
replaces 19.3 GFLOP of replicated latent-Q work per core. K/V and q_new
stay resident in SBUF; only the folded weights round-trip through DRAM.

Each core computes a partial out^T = (attn_out_g @ Wout[rows_g]).T for its
4 heads; the host sums the 4 partials per batch and transposes. bout is
added on-device by the g==0 cores only.
"""
import os
import sys

if "/opt/trn_rl_repo" not in sys.path:
    sys.path.insert(0, "/opt/trn_rl_repo")

import numpy as np

D_MODEL = 2048
Q_LAT = 1536
KV_LAT = 512
NUM_HEADS = 16
HD = 128
B, S = 2, 2048
SCALE = 1.0 / np.sqrt(2.0 * HD)  # 1/16

QT = 512          # query tile width (matmul free dim)
NQT = S // QT     # 4
NC_DM = D_MODEL // 128   # 16 chunks of the model dim
NC_QL = Q_LAT // 128     # 12
NC_KV = KV_LAT // 128    # 4
NKC = S // 128           # 16 key chunks

_CACHE = {}
LAST_RESULT = None


def _strip_cols(g):
    """Global column starts (width 128) of the 4 local head strips, in local
    order [2g, 2g+1, 2g+8, 2g+9]."""
    return [256 * g, 256 * g + 128, 1024 + 256 * g, 1024 + 256 * g + 128]


def _build_bass():
    from concourse import bacc, mybir
    from concourse.tile import TileContext

    f32 = mybir.dt.float32
    bf16 = mybir.dt.bfloat16
    AF = mybir.ActivationFunctionType

    nc = bacc.Bacc("TRN2", target_bir_lowering=False, debug=False)

    def inp(name, shape, dt=bf16):
        return nc.dram_tensor(name, list(shape), dt, kind="ExternalInput")

    xqT = inp("xqT", (D_MODEL, S))
    xkT = inp("xkT", (D_MODEL, S))
    # Wq_down^T tiles for the fold, c-major: [dm-chunk c][p=lat][l*128+f]
    wq_downT = inp("wq_downT", (NC_DM, 128, NC_QL * 128))
    wkv_down = inp("wkv_down", (NC_KV, 128, NC_DM * 128))  # [s][p=dm][c*128+f]
    wk_rope = inp("wk_rope", (128, NC_DM * 128))           # [p=dm][c*128+f]
    # up-proj slices for the fold: [p=lat][l(lat-chunk)*512 + f(4 strips x 128)]
    wq_up = inp("wq_up", (128, NC_QL * 512))
    wq_rope = inp("wq_rope", (128, NC_QL * 512))
    wk_up = inp("wk_up", (4, 128, NC_KV * 128))            # [strip][p=lat][c*128+f]
    wv_up = inp("wv_up", (128, NC_KV * 512))               # [p=lat][c*512+f]
    wout = inp("wout", (128, 64 * 128))                    # [p][(m*4+h)*128+f]
    cos_q = inp("cos_q", (2, 128, S), f32)                 # [block j][d][q]
    sin_q = inp("sin_q", (2, 128, S), f32)
    cos_k = inp("cos_k", (64, S))
    sin_k = inp("sin_k", (64, S))
    masks = inp("masks", (128, 4 * QT))                    # [kl][(o*QT)+ql]
    ones = inp("ones", (128, 128))
    bias = inp("bias", (128, NC_DM), f32)                  # [p][m]

    outT = nc.dram_tensor("outT", [D_MODEL, S], f32, kind="ExternalOutput")

    # folded Weff spill: [kind][dm-chunk c][p=dm][f=4 strips x 128]
    weff_d = nc.dram_tensor("weff_d", [2, NC_DM, 128, 512], bf16, kind="Internal")

    xqT_v = xqT.ap().rearrange("(c p) q -> p c q", p=128)  # [128, 16, 2048]
    xkT_v = xkT.ap().rearrange("(c p) q -> p c q", p=128)

    with TileContext(nc) as tc:
        with tc.tile_pool(name="kvres", bufs=1) as kvres, \
             tc.tile_pool(name="qnres", bufs=1) as qnres, \
             tc.tile_pool(name="xstream", bufs=2) as xstream:
            # resident outputs of phase A1 / A2 (consumed in phase B)
            kproj_sb = kvres.tile([128, 4, S], bf16)
            krope_sb = kvres.tile([128, S], bf16)
            v_sb = kvres.tile([128, NKC, 512], bf16)
            qn_sb = qnres.tile([128, 8, S], bf16)  # [2*strip + (0=proj,1=rope)]

            # ---------------- Phase F: fold Weff = Wq_down @ Wq_up|rope ----
            with tc.tile_pool(name="pfw", bufs=1) as pfw, \
                 tc.tile_pool(name="pfd", bufs=2) as pfd, \
                 tc.tile_pool(name="pfc", bufs=3) as pfc, \
                 tc.tile_pool(name="pfps", bufs=2, space="PSUM") as pfps:
                wqu_sb = pfw.tile([128, NC_QL * 512], bf16)
                wqr_sb = pfw.tile([128, NC_QL * 512], bf16)
                nc.sync.dma_start(out=wqu_sb, in_=wq_up.ap())
                nc.sync.dma_start(out=wqr_sb, in_=wq_rope.ap())
                for c in range(NC_DM):
                    wqdT_c = pfd.tile([128, NC_QL, 128], bf16, tag="wqdT")
                    nc.sync.dma_start(out=wqdT_c, in_=wq_downT.ap()[c])
                    for kind, wup in ((0, wqu_sb), (1, wqr_sb)):
                        ps = pfps.tile([128, 512], f32, tag="ps")
                        for l in range(NC_QL):
                            nc.tensor.matmul(
                                ps, wqdT_c[:, l, :], wup[:, l * 512:(l + 1) * 512],
                                start=(l == 0), stop=(l == NC_QL - 1))
                        cp = pfc.tile([128, 512], bf16, tag="cp")
                        nc.scalar.copy(out=cp, in_=ps)
                        nc.sync.dma_start(out=weff_d.ap()[kind][c], in_=cp)

            # ------------- Phase A1: K/V build (latkv, k_proj, V, k_rope) --
            with tc.tile_pool(name="a1w", bufs=1) as a1w, \
                 tc.tile_pool(name="a1t", bufs=2) as a1t, \
                 tc.tile_pool(name="a1ps", bufs=2, space="PSUM") as a1ps:
                wkv_sb = a1w.tile([128, NC_KV * NC_DM * 128], bf16)
                for s in range(NC_KV):
                    nc.sync.dma_start(
                        out=wkv_sb[:, s * NC_DM * 128:(s + 1) * NC_DM * 128],
                        in_=wkv_down.ap()[s])
                wkr_sb = a1w.tile([128, NC_DM * 128], bf16)
                nc.sync.dma_start(out=wkr_sb, in_=wk_rope.ap())
                wku_sb = a1w.tile([128, 4 * NC_KV * 128], bf16)
                for s in range(4):
                    nc.sync.dma_start(
                        out=wku_sb[:, s * NC_KV * 128:(s + 1) * NC_KV * 128],
                        in_=wk_up.ap()[s])
                wvu_sb = a1w.tile([128, NC_KV * 512], bf16)
                nc.sync.dma_start(out=wvu_sb, in_=wv_up.ap())
                cosk_sb = a1w.tile([64, S], bf16)
                sink_sb = a1w.tile([64, S], bf16)
                nc.sync.dma_start(out=cosk_sb, in_=cos_k.ap())
                nc.sync.dma_start(out=sink_sb, in_=sin_k.ap())

                for kt in range(4):  # k tiles of 512
                    k0 = kt * QT
                    xk_t = xstream.tile([128, NC_DM, QT], bf16, tag="x")
                    nc.sync.dma_start(out=xk_t, in_=xkT_v[:, :, k0:k0 + QT])
                    latkv = a1t.tile([128, NC_KV, QT], bf16, tag="latkv")
                    for s in range(NC_KV):
                        ps = a1ps.tile([128, QT], f32, tag="ps")
                        for c in range(NC_DM):
                            nc.tensor.matmul(
                                ps, wkv_sb[:, (s * NC_DM + c) * 128:(s * NC_DM + c + 1) * 128],
                                xk_t[:, c, :], start=(c == 0), stop=(c == NC_DM - 1))
                        nc.scalar.copy(out=latkv[:, s, :], in_=ps)
                    # k_rope raw
                    ps = a1ps.tile([128, QT], f32, tag="ps")
                    for c in range(NC_DM):
                        nc.tensor.matmul(
                            ps, wkr_sb[:, c * 128:(c + 1) * 128],
                            xk_t[:, c, :], start=(c == 0), stop=(c == NC_DM - 1))
                    krraw = a1t.tile([128, QT], f32, tag="krraw")
                    nc.scalar.copy(out=krraw, in_=ps)
                    krb = a1t.tile([64, QT], f32, tag="krb")
                    nc.sync.dma_start(out=krb, in_=krraw[64:128, :])
                    ck = cosk_sb[:, k0:k0 + QT]
                    sk = sink_sb[:, k0:k0 + QT]
                    t1 = a1t.tile([64, QT], f32, tag="krt1")
                    t2 = a1t.tile([64, QT], f32, tag="krt2")
                    nc.vector.tensor_mul(t1, krraw[0:64, :], ck)
                    nc.vector.tensor_mul(t2, krb, sk)
                    nc.vector.tensor_sub(krope_sb[0:64, k0:k0 + QT], t1, t2)
                    obot = a1t.tile([64, QT], bf16, tag="krob")
                    nc.vector.tensor_mul(t1, krb, ck)
                    nc.vector.tensor_mul(t2, krraw[0:64, :], sk)
                    nc.vector.tensor_add(obot, t1, t2)
                    nc.sync.dma_start(out=krope_sb[64:128, k0:k0 + QT], in_=obot)
                    # k_projT strips
                    for s in range(4):
                        ps = a1ps.tile([128, QT], f32, tag="ps")
                        for c in range(NC_KV):
                            nc.tensor.matmul(
                                ps, wku_sb[:, (s * NC_KV + c) * 128:(s * NC_KV + c + 1) * 128],
                                latkv[:, c, :], start=(c == 0), stop=(c == NC_KV - 1))
                        nc.scalar.copy(out=kproj_sb[:, s, k0:k0 + QT], in_=ps)
                    # V natural
                    for kc in range(4):  # 128-chunks within this k tile
                        ps = a1ps.tile([128, 512], f32, tag="ps")
                        for c in range(NC_KV):
                            nc.tensor.matmul(
                                ps, latkv[:, c, kc * 128:(kc + 1) * 128],
                                wvu_sb[:, c * 512:(c + 1) * 512],
                                start=(c == 0), stop=(c == NC_KV - 1))
                        nc.scalar.copy(out=v_sb[:, kt * 4 + kc, :], in_=ps)

            # ------------- Phase A2a: q_proj strips from x and Weff --------
            with tc.tile_pool(name="a2w", bufs=1) as a2w, \
                 tc.tile_pool(name="a2ps", bufs=2, space="PSUM") as a2ps:
                weffa_sb = a2w.tile([128, NC_DM, 512], bf16)
                nc.sync.dma_start(out=weffa_sb, in_=weff_d.ap().rearrange(
                    "k c p f -> p k c f")[:, 0])
                for qt in range(NQT):
                    q0 = qt * QT
                    xq_t = xstream.tile([128, NC_DM, QT], bf16, tag="x")
                    nc.sync.dma_start(out=xq_t, in_=xqT_v[:, :, q0:q0 + QT])
                    for s in range(4):
                        ps = a2ps.tile([128, QT], f32, tag="ps")
                        for c in range(NC_DM):
                            nc.tensor.matmul(
                                ps, weffa_sb[:, c, s * 128:(s + 1) * 128],
                                xq_t[:, c, :], start=(c == 0), stop=(c == NC_DM - 1))
                        nc.scalar.copy(out=qn_sb[:, 2 * s, q0:q0 + QT], in_=ps)

            # ------------- Phase A2b: q_rope strips from x and Weff --------
            with tc.tile_pool(name="a3w", bufs=1) as a3w, \
                 tc.tile_pool(name="a3t", bufs=2) as a3t, \
                 tc.tile_pool(name="a3ps", bufs=2, space="PSUM") as a3ps:
                weffb_sb = a3w.tile([128, NC_DM, 512], bf16)
                nc.sync.dma_start(out=weffb_sb, in_=weff_d.ap().rearrange(
                    "k c p f -> p k c f")[:, 1])
                cosq_sb = a3w.tile([128, 2, S], f32)
                sinq_sb = a3w.tile([128, 2, S], f32)
                for j in range(2):
                    nc.sync.dma_start(out=cosq_sb[:, j, :], in_=cos_q.ap()[j])
                    nc.sync.dma_start(out=sinq_sb[:, j, :], in_=sin_q.ap()[j])
                for qt in range(NQT):
                    q0 = qt * QT
                    xq_t = xstream.tile([128, NC_DM, QT], bf16, tag="x")
                    nc.sync.dma_start(out=xq_t, in_=xqT_v[:, :, q0:q0 + QT])
                    raw = []
                    for s in range(4):
                        ps = a3ps.tile([128, QT], f32, tag="ps")
                        for c in range(NC_DM):
                            nc.tensor.matmul(
                                ps, weffb_sb[:, c, s * 128:(s + 1) * 128],
                                xq_t[:, c, :], start=(c == 0), stop=(c == NC_DM - 1))
                        rw = a3t.tile([128, QT], f32, tag=f"raw{s}")
                        nc.scalar.copy(out=rw, in_=ps)
                        raw.append(rw)
                    for j in range(2):
                        a, b = raw[j], raw[2 + j]
                        cj = cosq_sb[:, j, q0:q0 + QT]
                        sj = sinq_sb[:, j, q0:q0 + QT]
                        t1 = a3t.tile([128, QT], f32, tag=f"t1{j}")
                        t2 = a3t.tile([128, QT], f32, tag=f"t2{j}")
                        nc.vector.tensor_mul(t1, a, cj)
                        nc.vector.tensor_mul(t2, b, sj)
                        nc.vector.tensor_sub(qn_sb[:, 2 * j + 1, q0:q0 + QT], t1, t2)
                        nc.vector.tensor_mul(t1, b, cj)
                        nc.vector.tensor_mul(t2, a, sj)
                        nc.vector.tensor_add(qn_sb[:, 2 * (2 + j) + 1, q0:q0 + QT], t1, t2)

            # ------------- Phase B: attention + output projection ----------
            with tc.tile_pool(name="bw", bufs=1) as bw, \
                 tc.tile_pool(name="be", bufs=4) as be, \
                 tc.tile_pool(name="ba", bufs=2) as ba, \
                 tc.tile_pool(name="bo", bufs=2) as bo, \
                 tc.tile_pool(name="bps", bufs=2, space="PSUM") as bps, \
                 tc.tile_pool(name="bpd", bufs=2, space="PSUM") as bpd, \
                 tc.tile_pool(name="bpv", bufs=2, space="PSUM") as bpv, \
                 tc.tile_pool(name="bpo", bufs=2, space="PSUM") as bpo:
                wout_sb = bw.tile([128, 64 * 128], bf16)
                nc.sync.dma_start(out=wout_sb, in_=wout.ap())
                masks_sb = bw.tile([128, 4 * QT], bf16)
                nc.sync.dma_start(out=masks_sb, in_=masks.ap())
                ones_sb = bw.tile([128, 128], bf16)
                nc.sync.dma_start(out=ones_sb, in_=ones.ap())
                bias_sb = bw.tile([128, NC_DM], f32)
                nc.sync.dma_start(out=bias_sb, in_=bias.ap())

                for qt in range(NQT):
                    q0 = qt * QT
                    K = (q0 + QT) // 128  # causal: chunks 0..K-1
                    attn = ba.tile([128, 4, QT], bf16, tag="attn")
                    for h in range(4):
                        psd = bpd.tile([128, QT], f32, tag="psd")
                        psv = bpv.tile([128, QT], f32, tag="psv")
                        for kc in range(K):
                            pss = bps.tile([128, QT], f32, tag="pss")
                            nc.tensor.matmul(
                                pss, kproj_sb[:, h, kc * 128:(kc + 1) * 128],
                                qn_sb[:, 2 * h, q0:q0 + QT], start=True, stop=False)
                            nc.tensor.matmul(
                                pss, krope_sb[:, kc * 128:(kc + 1) * 128],
                                qn_sb[:, 2 * h + 1, q0:q0 + QT], start=False, stop=True)
                            ex = be.tile([128, QT], bf16, tag="ex")
                            nc.scalar.activation(out=ex, in_=pss, func=AF.Exp,
                                                 scale=float(SCALE))
                            o = kc - q0 // 128
                            if o >= 0:  # diagonal chunk: apply causal mask
                                nc.vector.tensor_mul(
                                    ex, ex, masks_sb[:, o * QT:(o + 1) * QT])
                            nc.tensor.matmul(
                                psd, ones_sb, ex,
                                start=(kc == 0), stop=(kc == K - 1),
                                skip_group_check=True)
                            nc.tensor.matmul(
                                psv, v_sb[:, kc, h * 128:(h + 1) * 128], ex,
                                start=(kc == 0), stop=(kc == K - 1),
                                skip_group_check=True)
                        rec = be.tile([128, QT], f32, tag="rec")
                        nc.vector.reciprocal_approx_fast(out=rec, in_=psd)
                        nc.vector.tensor_mul(attn[:, h, :], psv, rec)
                    # output projection for this q tile
                    for m in range(NC_DM):
                        pso = bpo.tile([128, QT], f32, tag="pso")
                        for h in range(4):
                            nc.tensor.matmul(
                                pso, wout_sb[:, (m * 4 + h) * 128:(m * 4 + h + 1) * 128],
                                attn[:, h, :], start=(h == 0), stop=(h == 3))
                        oc = bo.tile([128, QT], f32, tag="oc")
                        nc.scalar.activation(
                            out=oc, in_=pso, func=AF.Identity,
                            bias=bias_sb[:, m:m + 1], scale=1.0)
                        nc.sync.dma_start(
                            out=outT.ap()[m * 128:(m + 1) * 128, q0:q0 + QT], in_=oc)

    nc.finalize()
    return nc


def _host_pack(inputs):
    """Build the 8 per-core input maps from the full inputs."""
    import ml_dtypes
    bf16 = ml_dtypes.bfloat16

    xq = np.ascontiguousarray(inputs["inputs_q"], dtype=np.float32)
    xk = np.ascontiguousarray(inputs["inputs_k"], dtype=np.float32)
    Wq_down = np.asarray(inputs["Wq_down"], dtype=np.float32)
    Wkv_down = np.asarray(inputs["Wkv_down"], dtype=np.float32)
    Wq_up = np.asarray(inputs["Wq_up"], dtype=np.float32)
    Wk_up = np.asarray(inputs["Wk_up"], dtype=np.float32)
    Wv_up = np.asarray(inputs["Wv_up"], dtype=np.float32)
    Wq_rope = np.asarray(inputs["Wq_rope"], dtype=np.float32)
    Wk_rope = np.asarray(inputs["Wk_rope"], dtype=np.float32)
    Wout = np.asarray(inputs["Wout"], dtype=np.float32)
    bout = np.asarray(inputs["bout"], dtype=np.float32)

    def pack_lhs(W, n_strips, strip_starts, nchunks):
        # -> [n_strips, 128, nchunks*128]: [s][p][c*128+f]
        out = np.empty((n_strips, 128, nchunks * 128), dtype=bf16)
        for s in range(n_strips):
            blk = W[:, strip_starts[s]:strip_starts[s] + 128]  # [nchunks*128, 128]
            out[s] = blk.reshape(nchunks, 128, 128).transpose(1, 0, 2).reshape(128, -1).astype(bf16)
        return out

    xqT = [np.ascontiguousarray(xq[b].T.astype(bf16)) for b in range(B)]
    xkT = [np.ascontiguousarray(xk[b].T.astype(bf16)) for b in range(B)]

    # Wq_down^T packed c-major for the fold: [c][p=lat][l*128+f(dm)]
    WqdT = np.ascontiguousarray(Wq_down.T)  # [Q_LAT, D_MODEL]
    wq_downT_p = np.ascontiguousarray(
        WqdT.reshape(NC_QL, 128, NC_DM, 128).transpose(2, 1, 0, 3)
        .reshape(NC_DM, 128, NC_QL * 128).astype(bf16))

    wkv_down_p = pack_lhs(Wkv_down, NC_KV, [128 * s for s in range(NC_KV)], NC_DM)
    wk_rope_p = pack_lhs(Wk_rope, 1, [0], NC_DM)[0]

    # rope tables (fp32)
    iq = np.arange(1024, dtype=np.float64)
    inv_q = 1.0 / (10000.0 ** (iq * 2.0 / D_MODEL))
    pos = np.arange(S, dtype=np.float64)
    ang_q = pos[:, None] * inv_q[None, :]          # [S, 1024]
    ik = np.arange(64, dtype=np.float64)
    inv_k = 1.0 / (10000.0 ** (ik * 2.0 / HD))
    ang_k = pos[:, None] * inv_k[None, :]          # [S, 64]
    cos_k = np.ascontiguousarray(np.cos(ang_k).T.astype(bf16))  # [64, S]
    sin_k = np.ascontiguousarray(np.sin(ang_k).T.astype(bf16))

    kl = np.arange(128)[:, None]
    ql = np.arange(QT)[None, :]
    masks = np.concatenate(
        [(kl + 128 * o <= ql).astype(np.float32) for o in range(4)], axis=1)
    masks = np.ascontiguousarray(masks.astype(bf16))
    ones = np.ones((128, 128), dtype=bf16)

    in_maps = []
    for c in range(8):
        b, g = divmod(c, 4)
        cols = _strip_cols(g)
        cols4 = np.concatenate([np.arange(cs, cs + 128) for cs in cols])

        # up-proj slices for fold rhs: [p=lat within chunk][l*512 + f]
        def pack_up(W):
            Wg = W[:, cols4]  # [Q_LAT, 512]
            return np.ascontiguousarray(
                Wg.reshape(NC_QL, 128, 512).transpose(1, 0, 2).reshape(128, -1).astype(bf16))
        wq_up_p = pack_up(Wq_up)
        wq_rope_p = pack_up(Wq_rope)
        wk_up_p = pack_lhs(Wk_up, 4, cols, NC_KV)
        Wv_g = Wv_up[:, cols4]                      # [512, 512]
        wv_up_p = np.ascontiguousarray(
            Wv_g.reshape(NC_KV, 128, 512).transpose(1, 0, 2).reshape(128, -1).astype(bf16))
        Wout_g = Wout[cols4, :].reshape(4, 128, NC_DM, 128)   # [h][p][m][f]
        wout_p = np.ascontiguousarray(
            Wout_g.transpose(1, 2, 0, 3).reshape(128, -1).astype(bf16))
        cos_q_p = np.empty((2, 128, S), dtype=np.float32)
        sin_q_p = np.empty((2, 128, S), dtype=np.float32)
        for j in range(2):
            idx = 256 * g + 128 * j + np.arange(128)
            cos_q_p[j] = np.cos(ang_q[:, idx]).T
            sin_q_p[j] = np.sin(ang_q[:, idx]).T
        bias_p = (bout if g == 0 else np.zeros_like(bout)).reshape(NC_DM, 128)
        bias_p = np.ascontiguousarray(bias_p.T)     # [128, m]

        in_maps.append({
            "xqT": xqT[b], "xkT": xkT[b],
            "wq_downT": wq_downT_p, "wkv_down": wkv_down_p, "wk_rope": wk_rope_p,
            "wq_up": wq_up_p, "wq_rope": wq_rope_p, "wk_up": wk_up_p,
            "wv_up": wv_up_p, "wout": wout_p,
            "cos_q": cos_q_p, "sin_q": sin_q_p, "cos_k": cos_k, "sin_k": sin_k,
            "masks": masks, "ones": ones, "bias": bias_p,
        })
    return in_maps


def kernel(**inputs):
    global LAST_RESULT
    from concourse.bass_utils import run_bass_kernel_spmd

    if "nc" not in _CACHE:
        _CACHE["nc"] = _build_bass()
    nc = _CACHE["nc"]

    in_maps = _host_pack(inputs)
    kwargs = {}
    if os.environ.get("KERNEL_TRACE"):
        try:
            sys.path.insert(0, os.path.dirname(os.path.abspath(__file__)))
            import axon_shim
            axon_shim.install()
        except Exception:
            pass
        kwargs["trace"] = True
    res = run_bass_kernel_spmd(nc, in_maps, core_ids=list(range(8)), **kwargs)
    LAST_RESULT = res

    out = np.empty((B, S, D_MODEL), dtype=np.float32)
    for b in range(B):
        acc = res.results[4 * b]["outT"].copy()
        for g in range(1, 4):
            acc += res.results[4 * b + g]["outT"]
        out[b] = acc.T
    return out


# revision 7
# speedup vs baseline: 1.5868x; 1.0091x over previous
"""MLA (Multi-Head Latent Attention) Bass kernel for 8 Trainium2 NeuronCores.

Sharding: 8 cores = 2 (batch) x 4 (head groups). Core c -> batch c//4,
group g=c%4 owning heads {2g, 2g+1, 2g+8, 2g+9} (paired h/h+8 so the
rotate-half RoPE over d_model=2048 stays core-local).

All activations flow on-device in transposed [feature, token] layout so no
on-chip transposes are needed (the host pre-transposes x). Attention scores
are computed in [k, q] layout; the softmax denominator is computed with an
all-ones matmul on the PE (scores are bounded, so no max subtraction), exp
runs on the scalar engine straight out of PSUM, and 1/denom is folded into
the attention-output scaling.

Matmuls run in bf16 (fp32 PSUM accumulation). The core folds its slice of
Wq_down @ Wq_up (and @ Wq_rope) on-device first — 6.4 GFLOP of folding
replaces 19.3 GFLOP of replicated latent-Q work per core. K/V and q_new
stay resident in SBUF; only the folded weights round-trip through DRAM.

Each core computes a partial out^T = (attn_out_g @ Wout[rows_g]).T for its
4 heads; the host sums the 4 partials per batch and transposes. bout is
added on-device by the g==0 cores only.
"""
import os
import sys

if "/opt/trn_rl_repo" not in sys.path:
    sys.path.insert(0, "/opt/trn_rl_repo")

import numpy as np

D_MODEL = 2048
Q_LAT = 1536
KV_LAT = 512
NUM_HEADS = 16
HD = 128
B, S = 2, 2048
SCALE = 1.0 / np.sqrt(2.0 * HD)  # 1/16

QT = 512          # query tile width (matmul free dim)
NQT = S // QT     # 4
NC_DM = D_MODEL // 128   # 16 chunks of the model dim
NC_QL = Q_LAT // 128     # 12
NC_KV = KV_LAT // 128    # 4
NKC = S // 128           # 16 key chunks

_CACHE = {}
LAST_RESULT = None


def _strip_cols(g):
    """Global column starts (width 128) of the 4 local head strips, in local
    order [2g, 2g+1, 2g+8, 2g+9]."""
    return [256 * g, 256 * g + 128, 1024 + 256 * g, 1024 + 256 * g + 128]


def _build_bass():
    from concourse import bacc, mybir
    from concourse.tile import TileContext

    f32 = mybir.dt.float32
    bf16 = mybir.dt.bfloat16
    AF = mybir.ActivationFunctionType

    nc = bacc.Bacc("TRN2", target_bir_lowering=False, debug=False)

    def inp(name, shape, dt=bf16):
        return nc.dram_tensor(name, list(shape), dt, kind="ExternalInput")

    xqT = inp("xqT", (D_MODEL, S))
    xkT = inp("xkT", (D_MODEL, S))
    # Wq_down^T tiles for the fold, c-major: [dm-chunk c][p=lat][l*128+f]
    wq_downT = inp("wq_downT", (NC_DM, 128, NC_QL * 128))
    wkv_down = inp("wkv_down", (NC_KV, 128, NC_DM * 128))  # [s][p=dm][c*128+f]
    wk_rope = inp("wk_rope", (128, NC_DM * 128))           # [p=dm][c*128+f]
    # up-proj slices for the fold: [p=lat][l(lat-chunk)*512 + f(4 strips x 128)]
    wq_up = inp("wq_up", (128, NC_QL * 512))
    wq_rope = inp("wq_rope", (128, NC_QL * 512))
    wk_up = inp("wk_up", (4, 128, NC_KV * 128))            # [strip][p=lat][c*128+f]
    wv_up = inp("wv_up", (128, NC_KV * 512))               # [p=lat][c*512+f]
    wout = inp("wout", (128, 64 * 128))                    # [p][(m*4+h)*128+f]
    cos_q = inp("cos_q", (2, 128, S), f32)                 # [block j][d][q]
    sin_q = inp("sin_q", (2, 128, S), f32)
    cos_k = inp("cos_k", (64, S))
    sin_k = inp("sin_k", (64, S))
    masks = inp("masks", (128, 4 * QT))                    # [kl][(o*QT)+ql]
    ones = inp("ones", (128, 128))
    bias = inp("bias", (128, NC_DM), f32)                  # [p][m]

    outT = nc.dram_tensor("outT", [D_MODEL, S], f32, kind="ExternalOutput")

    # folded Weff spill: [kind][dm-chunk c][p=dm][f=4 strips x 128]
    weff_d = nc.dram_tensor("weff_d", [2, NC_DM, 128, 512], bf16, kind="Internal")

    xqT_v = xqT.ap().rearrange("(c p) q -> p c q", p=128)  # [128, 16, 2048]
    xkT_v = xkT.ap().rearrange("(c p) q -> p c q", p=128)

    with TileContext(nc) as tc:
        with tc.tile_pool(name="kvres", bufs=1) as kvres, \
             tc.tile_pool(name="qnres", bufs=1) as qnres, \
             tc.tile_pool(name="xstream", bufs=2) as xstream:
            # resident outputs of phase A1 / A2 (consumed in phase B)
            kproj_sb = kvres.tile([128, 4, S], bf16)
            krope_sb = kvres.tile([128, S], bf16)
            v_sb = kvres.tile([128, NKC, 512], bf16)
            qn_sb = qnres.tile([128, 8, S], bf16)  # [2*strip + (0=proj,1=rope)]

            # ---------------- Phase F: fold Weff = Wq_down @ Wq_up|rope ----
            with tc.tile_pool(name="pfw", bufs=1) as pfw, \
                 tc.tile_pool(name="pfd", bufs=2) as pfd, \
                 tc.tile_pool(name="pfc", bufs=3) as pfc, \
                 tc.tile_pool(name="pfps", bufs=2, space="PSUM") as pfps:
                wqu_sb = pfw.tile([128, NC_QL * 512], bf16)
                wqr_sb = pfw.tile([128, NC_QL * 512], bf16)
                nc.sync.dma_start(out=wqu_sb, in_=wq_up.ap())
                nc.sync.dma_start(out=wqr_sb, in_=wq_rope.ap())
                for c in range(NC_DM):
                    wqdT_c = pfd.tile([128, NC_QL, 128], bf16, tag="wqdT")
                    nc.sync.dma_start(out=wqdT_c, in_=wq_downT.ap()[c])
                    for kind, wup in ((0, wqu_sb), (1, wqr_sb)):
                        ps = pfps.tile([128, 512], f32, tag="ps")
                        for l in range(NC_QL):
                            nc.tensor.matmul(
                                ps, wqdT_c[:, l, :], wup[:, l * 512:(l + 1) * 512],
                                start=(l == 0), stop=(l == NC_QL - 1))
                        cp = pfc.tile([128, 512], bf16, tag="cp")
                        nc.scalar.copy(out=cp, in_=ps)
                        nc.sync.dma_start(out=weff_d.ap()[kind][c], in_=cp)

            # ------------- Phase A1: K/V build (latkv, k_proj, V, k_rope) --
            with tc.tile_pool(name="a1w", bufs=1) as a1w, \
                 tc.tile_pool(name="a1t", bufs=2) as a1t, \
                 tc.tile_pool(name="a1ps", bufs=2, space="PSUM") as a1ps:
                wkv_sb = a1w.tile([128, NC_KV * NC_DM * 128], bf16)
                for s in range(NC_KV):
                    nc.sync.dma_start(
                        out=wkv_sb[:, s * NC_DM * 128:(s + 1) * NC_DM * 128],
                        in_=wkv_down.ap()[s])
                wkr_sb = a1w.tile([128, NC_DM * 128], bf16)
                nc.sync.dma_start(out=wkr_sb, in_=wk_rope.ap())
                wku_sb = a1w.tile([128, 4 * NC_KV * 128], bf16)
                for s in range(4):
                    nc.sync.dma_start(
                        out=wku_sb[:, s * NC_KV * 128:(s + 1) * NC_KV * 128],
                        in_=wk_up.ap()[s])
                wvu_sb = a1w.tile([128, NC_KV * 512], bf16)
                nc.sync.dma_start(out=wvu_sb, in_=wv_up.ap())
                cosk_sb = a1w.tile([64, S], bf16)
                sink_sb = a1w.tile([64, S], bf16)
                nc.sync.dma_start(out=cosk_sb, in_=cos_k.ap())
                nc.sync.dma_start(out=sink_sb, in_=sin_k.ap())

                for kt in range(4):  # k tiles of 512
                    k0 = kt * QT
                    xk_t = xstream.tile([128, NC_DM, QT], bf16, tag="x")
                    nc.sync.dma_start(out=xk_t, in_=xkT_v[:, :, k0:k0 + QT])
                    latkv = a1t.tile([128, NC_KV, QT], bf16, tag="latkv")
                    for s in range(NC_KV):
                        ps = a1ps.tile([128, QT], f32, tag="ps")
                        for c in range(NC_DM):
                            nc.tensor.matmul(
                                ps, wkv_sb[:, (s * NC_DM + c) * 128:(s * NC_DM + c + 1) * 128],
                                xk_t[:, c, :], start=(c == 0), stop=(c == NC_DM - 1))
                        nc.scalar.copy(out=latkv[:, s, :], in_=ps)
                    # k_rope raw
                    ps = a1ps.tile([128, QT], f32, tag="ps")
                    for c in range(NC_DM):
                        nc.tensor.matmul(
                            ps, wkr_sb[:, c * 128:(c + 1) * 128],
                            xk_t[:, c, :], start=(c == 0), stop=(c == NC_DM - 1))
                    krraw = a1t.tile([128, QT], f32, tag="krraw")
                    nc.scalar.copy(out=krraw, in_=ps)
                    krb = a1t.tile([64, QT], f32, tag="krb")
                    nc.sync.dma_start(out=krb, in_=krraw[64:128, :])
                    ck = cosk_sb[:, k0:k0 + QT]
                    sk = sink_sb[:, k0:k0 + QT]
                    t1 = a1t.tile([64, QT], f32, tag="krt1")
                    t2 = a1t.tile([64, QT], f32, tag="krt2")
                    nc.vector.tensor_mul(t1, krraw[0:64, :], ck)
                    nc.vector.tensor_mul(t2, krb, sk)
                    nc.vector.tensor_sub(krope_sb[0:64, k0:k0 + QT], t1, t2)
                    obot = a1t.tile([64, QT], bf16, tag="krob")
                    nc.vector.tensor_mul(t1, krb, ck)
                    nc.vector.tensor_mul(t2, krraw[0:64, :], sk)
                    nc.vector.tensor_add(obot, t1, t2)
                    nc.sync.dma_start(out=krope_sb[64:128, k0:k0 + QT], in_=obot)
                    # k_projT strips
                    for s in range(4):
                        ps = a1ps.tile([128, QT], f32, tag="ps")
                        for c in range(NC_KV):
                            nc.tensor.matmul(
                                ps, wku_sb[:, (s * NC_KV + c) * 128:(s * NC_KV + c + 1) * 128],
                                latkv[:, c, :], start=(c == 0), stop=(c == NC_KV - 1))
                        nc.scalar.copy(out=kproj_sb[:, s, k0:k0 + QT], in_=ps)
                    # V natural
                    for kc in range(4):  # 128-chunks within this k tile
                        ps = a1ps.tile([128, 512], f32, tag="ps")
                        for c in range(NC_KV):
                            nc.tensor.matmul(
                                ps, latkv[:, c, kc * 128:(kc + 1) * 128],
                                wvu_sb[:, c * 512:(c + 1) * 512],
                                start=(c == 0), stop=(c == NC_KV - 1))
                        nc.scalar.copy(out=v_sb[:, kt * 4 + kc, :], in_=ps)

            # ------------- Phase A2: q_proj + q_rope strips from x ---------
            with tc.tile_pool(name="a2w", bufs=1) as a2w, \
                 tc.tile_pool(name="a2t", bufs=2) as a2t, \
                 tc.tile_pool(name="a2ps", bufs=2, space="PSUM") as a2ps:
                weffa_sb = a2w.tile([128, NC_DM, 512], bf16)
                nc.sync.dma_start(out=weffa_sb, in_=weff_d.ap().rearrange(
                    "k c p f -> p k c f")[:, 0])
                weffb_sb = a2w.tile([128, NC_DM, 512], bf16)
                nc.sync.dma_start(out=weffb_sb, in_=weff_d.ap().rearrange(
                    "k c p f -> p k c f")[:, 1])
                cosq_sb = a2w.tile([128, 2, S], f32)
                sinq_sb = a2w.tile([128, 2, S], f32)
                for j in range(2):
                    nc.sync.dma_start(out=cosq_sb[:, j, :], in_=cos_q.ap()[j])
                    nc.sync.dma_start(out=sinq_sb[:, j, :], in_=sin_q.ap()[j])
                for qt in range(NQT):
                    q0 = qt * QT
                    xq_t = xstream.tile([128, NC_DM, QT], bf16, tag="x")
                    nc.sync.dma_start(out=xq_t, in_=xqT_v[:, :, q0:q0 + QT])
                    for s in range(4):
                        ps = a2ps.tile([128, QT], f32, tag="ps")
                        for c in range(NC_DM):
                            nc.tensor.matmul(
                                ps, weffa_sb[:, c, s * 128:(s + 1) * 128],
                                xq_t[:, c, :], start=(c == 0), stop=(c == NC_DM - 1))
                        nc.scalar.copy(out=qn_sb[:, 2 * s, q0:q0 + QT], in_=ps)
                    raw = []
                    for s in range(4):
                        ps = a2ps.tile([128, QT], f32, tag="ps")
                        for c in range(NC_DM):
                            nc.tensor.matmul(
                                ps, weffb_sb[:, c, s * 128:(s + 1) * 128],
                                xq_t[:, c, :], start=(c == 0), stop=(c == NC_DM - 1))
                        rw = a2t.tile([128, QT], f32, tag=f"raw{s}")
                        nc.scalar.copy(out=rw, in_=ps)
                        raw.append(rw)
                    for j in range(2):
                        a, b = raw[j], raw[2 + j]
                        cj = cosq_sb[:, j, q0:q0 + QT]
                        sj = sinq_sb[:, j, q0:q0 + QT]
                        t1 = a2t.tile([128, QT], f32, tag=f"t1{j}")
                        t2 = a2t.tile([128, QT], f32, tag=f"t2{j}")
                        nc.vector.tensor_mul(t1, a, cj)
                        nc.vector.tensor_mul(t2, b, sj)
                        nc.vector.tensor_sub(qn_sb[:, 2 * j + 1, q0:q0 + QT], t1, t2)
                        nc.vector.tensor_mul(t1, b, cj)
                        nc.vector.tensor_mul(t2, a, sj)
                        nc.vector.tensor_add(qn_sb[:, 2 * (2 + j) + 1, q0:q0 + QT], t1, t2)

            # ------------- Phase B: attention + output projection ----------
            with tc.tile_pool(name="bw", bufs=1) as bw, \
                 tc.tile_pool(name="be", bufs=4) as be, \
                 tc.tile_pool(name="ba", bufs=2) as ba, \
                 tc.tile_pool(name="bo", bufs=2) as bo, \
                 tc.tile_pool(name="bps", bufs=2, space="PSUM") as bps, \
                 tc.tile_pool(name="bpd", bufs=2, space="PSUM") as bpd, \
                 tc.tile_pool(name="bpv", bufs=2, space="PSUM") as bpv, \
                 tc.tile_pool(name="bpo", bufs=2, space="PSUM") as bpo:
                wout_sb = bw.tile([128, 64 * 128], bf16)
                nc.sync.dma_start(out=wout_sb, in_=wout.ap())
                masks_sb = bw.tile([128, 4 * QT], bf16)
                nc.sync.dma_start(out=masks_sb, in_=masks.ap())
                ones_sb = bw.tile([128, 128], bf16)
                nc.sync.dma_start(out=ones_sb, in_=ones.ap())
                bias_sb = bw.tile([128, NC_DM], f32)
                nc.sync.dma_start(out=bias_sb, in_=bias.ap())

                for qt in range(NQT):
                    q0 = qt * QT
                    K = (q0 + QT) // 128  # causal: chunks 0..K-1
                    attn = ba.tile([128, 4, QT], bf16, tag="attn")
                    for h in range(4):
                        psd = bpd.tile([128, QT], f32, tag="psd")
                        psv = bpv.tile([128, QT], f32, tag="psv")
                        for kc in range(K):
                            pss = bps.tile([128, QT], f32, tag="pss")
                            nc.tensor.matmul(
                                pss, kproj_sb[:, h, kc * 128:(kc + 1) * 128],
                                qn_sb[:, 2 * h, q0:q0 + QT], start=True, stop=False)
                            nc.tensor.matmul(
                                pss, krope_sb[:, kc * 128:(kc + 1) * 128],
                                qn_sb[:, 2 * h + 1, q0:q0 + QT], start=False, stop=True)
                            ex = be.tile([128, QT], bf16, tag="ex")
                            nc.scalar.activation(out=ex, in_=pss, func=AF.Exp,
                                                 scale=float(SCALE))
                            o = kc - q0 // 128
                            if o >= 0:  # diagonal chunk: apply causal mask
                                nc.vector.tensor_mul(
                                    ex, ex, masks_sb[:, o * QT:(o + 1) * QT])
                            nc.tensor.matmul(
                                psd, ones_sb, ex,
                                start=(kc == 0), stop=(kc == K - 1),
                                skip_group_check=True)
                            nc.tensor.matmul(
                                psv, v_sb[:, kc, h * 128:(h + 1) * 128], ex,
                                start=(kc == 0), stop=(kc == K - 1),
                                skip_group_check=True)
                        rec = be.tile([128, QT], f32, tag="rec")
                        nc.vector.reciprocal_approx_fast(out=rec, in_=psd)
                        nc.vector.tensor_mul(attn[:, h, :], psv, rec)
                    # output projection for this q tile
                    for m in range(NC_DM):
                        pso = bpo.tile([128, QT], f32, tag="pso")
                        for h in range(4):
                            nc.tensor.matmul(
                                pso, wout_sb[:, (m * 4 + h) * 128:(m * 4 + h + 1) * 128],
                                attn[:, h, :], start=(h == 0), stop=(h == 3))
                        oc = bo.tile([128, QT], f32, tag="oc")
                        nc.scalar.activation(
                            out=oc, in_=pso, func=AF.Identity,
                            bias=bias_sb[:, m:m + 1], scale=1.0)
                        nc.sync.dma_start(
                            out=outT.ap()[m * 128:(m + 1) * 128, q0:q0 + QT], in_=oc)

    nc.finalize()
    return nc


def _host_pack(inputs):
    """Build the 8 per-core input maps from the full inputs."""
    import ml_dtypes
    bf16 = ml_dtypes.bfloat16

    xq = np.ascontiguousarray(inputs["inputs_q"], dtype=np.float32)
    xk = np.ascontiguousarray(inputs["inputs_k"], dtype=np.float32)
    Wq_down = np.asarray(inputs["Wq_down"], dtype=np.float32)
    Wkv_down = np.asarray(inputs["Wkv_down"], dtype=np.float32)
    Wq_up = np.asarray(inputs["Wq_up"], dtype=np.float32)
    Wk_up = np.asarray(inputs["Wk_up"], dtype=np.float32)
    Wv_up = np.asarray(inputs["Wv_up"], dtype=np.float32)
    Wq_rope = np.asarray(inputs["Wq_rope"], dtype=np.float32)
    Wk_rope = np.asarray(inputs["Wk_rope"], dtype=np.float32)
    Wout = np.asarray(inputs["Wout"], dtype=np.float32)
    bout = np.asarray(inputs["bout"], dtype=np.float32)

    def pack_lhs(W, n_strips, strip_starts, nchunks):
        # -> [n_strips, 128, nchunks*128]: [s][p][c*128+f]
        out = np.empty((n_strips, 128, nchunks * 128), dtype=bf16)
        for s in range(n_strips):
            blk = W[:, strip_starts[s]:strip_starts[s] + 128]  # [nchunks*128, 128]
            out[s] = blk.reshape(nchunks, 128, 128).transpose(1, 0, 2).reshape(128, -1).astype(bf16)
        return out

    xqT = [np.ascontiguousarray(xq[b].T.astype(bf16)) for b in range(B)]
    xkT = [np.ascontiguousarray(xk[b].T.astype(bf16)) for b in range(B)]

    # Wq_down^T packed c-major for the fold: [c][p=lat][l*128+f(dm)]
    WqdT = np.ascontiguousarray(Wq_down.T)  # [Q_LAT, D_MODEL]
    wq_downT_p = np.ascontiguousarray(
        WqdT.reshape(NC_QL, 128, NC_DM, 128).transpose(2, 1, 0, 3)
        .reshape(NC_DM, 128, NC_QL * 128).astype(bf16))

    wkv_down_p = pack_lhs(Wkv_down, NC_KV, [128 * s for s in range(NC_KV)], NC_DM)
    wk_rope_p = pack_lhs(Wk_rope, 1, [0], NC_DM)[0]

    # rope tables (fp32)
    iq = np.arange(1024, dtype=np.float64)
    inv_q = 1.0 / (10000.0 ** (iq * 2.0 / D_MODEL))
    pos = np.arange(S, dtype=np.float64)
    ang_q = pos[:, None] * inv_q[None, :]          # [S, 1024]
    ik = np.arange(64, dtype=np.float64)
    inv_k = 1.0 / (10000.0 ** (ik * 2.0 / HD))
    ang_k = pos[:, None] * inv_k[None, :]          # [S, 64]
    cos_k = np.ascontiguousarray(np.cos(ang_k).T.astype(bf16))  # [64, S]
    sin_k = np.ascontiguousarray(np.sin(ang_k).T.astype(bf16))

    kl = np.arange(128)[:, None]
    ql = np.arange(QT)[None, :]
    masks = np.concatenate(
        [(kl + 128 * o <= ql).astype(np.float32) for o in range(4)], axis=1)
    masks = np.ascontiguousarray(masks.astype(bf16))
    ones = np.ones((128, 128), dtype=bf16)

    in_maps = []
    for c in range(8):
        b, g = divmod(c, 4)
        cols = _strip_cols(g)
        cols4 = np.concatenate([np.arange(cs, cs + 128) for cs in cols])

        # up-proj slices for fold rhs: [p=lat within chunk][l*512 + f]
        def pack_up(W):
            Wg = W[:, cols4]  # [Q_LAT, 512]
            return np.ascontiguousarray(
                Wg.reshape(NC_QL, 128, 512).transpose(1, 0, 2).reshape(128, -1).astype(bf16))
        wq_up_p = pack_up(Wq_up)
        wq_rope_p = pack_up(Wq_rope)
        wk_up_p = pack_lhs(Wk_up, 4, cols, NC_KV)
        Wv_g = Wv_up[:, cols4]                      # [512, 512]
        wv_up_p = np.ascontiguousarray(
            Wv_g.reshape(NC_KV, 128, 512).transpose(1, 0, 2).reshape(128, -1).astype(bf16))
        Wout_g = Wout[cols4, :].reshape(4, 128, NC_DM, 128)   # [h][p][m][f]
        wout_p = np.ascontiguousarray(
            Wout_g.transpose(1, 2, 0, 3).reshape(128, -1).astype(bf16))
        cos_q_p = np.empty((2, 128, S), dtype=np.float32)
        sin_q_p = np.empty((2, 128, S), dtype=np.float32)
        for j in range(2):
            idx = 256 * g + 128 * j + np.arange(128)
            cos_q_p[j] = np.cos(ang_q[:, idx]).T
            sin_q_p[j] = np.sin(ang_q[:, idx]).T
        bias_p = (bout if g == 0 else np.zeros_like(bout)).reshape(NC_DM, 128)
        bias_p = np.ascontiguousarray(bias_p.T)     # [128, m]

        in_maps.append({
            "xqT": xqT[b], "xkT": xkT[b],
            "wq_downT": wq_downT_p, "wkv_down": wkv_down_p, "wk_rope": wk_rope_p,
            "wq_up": wq_up_p, "wq_rope": wq_rope_p, "wk_up": wk_up_p,
            "wv_up": wv_up_p, "wout": wout_p,
            "cos_q": cos_q_p, "sin_q": sin_q_p, "cos_k": cos_k, "sin_k": sin_k,
            "masks": masks, "ones": ones, "bias": bias_p,
        })
    return in_maps


def kernel(**inputs):
    global LAST_RESULT
    from concourse.bass_utils import run_bass_kernel_spmd

    if "nc" not in _CACHE:
        _CACHE["nc"] = _build_bass()
    nc = _CACHE["nc"]

    in_maps = _host_pack(inputs)
    kwargs = {}
    if os.environ.get("KERNEL_TRACE"):
        try:
            sys.path.insert(0, os.path.dirname(os.path.abspath(__file__)))
            import axon_shim
            axon_shim.install()
        except Exception:
            pass
        kwargs["trace"] = True
    res = run_bass_kernel_spmd(nc, in_maps, core_ids=list(range(8)), **kwargs)
    LAST_RESULT = res

    out = np.empty((B, S, D_MODEL), dtype=np.float32)
    for b in range(B):
        acc = res.results[4 * b]["outT"].copy()
        for g in range(1, 4):
            acc += res.results[4 * b + g]["outT"]
        out[b] = acc.T
    return out


# revision 9
# speedup vs baseline: 1.6208x; 1.0215x over previous
"""MLA (Multi-Head Latent Attention) Bass kernel for 8 Trainium2 NeuronCores.

Sharding: 8 cores = 2 (batch) x 4 (head groups). Core c -> batch c//4,
group g=c%4 owning heads {2g, 2g+1, 2g+8, 2g+9} (paired h/h+8 so the
rotate-half RoPE over d_model=2048 stays core-local).

All activations flow on-device in transposed [feature, token] layout so no
on-chip transposes are needed (the host pre-transposes x). Attention scores
are computed in [k, q] layout; the softmax denominator is computed with an
all-ones matmul on the PE (scores are bounded, so no max subtraction), exp
runs on the scalar engine straight out of PSUM, and 1/denom is folded into
the attention-output scaling.

Matmuls run in bf16 (fp32 PSUM accumulation). The core folds its slice of
Wq_down @ Wq_up (and @ Wq_rope) on-device first — 6.4 GFLOP of folding
replaces 19.3 GFLOP of replicated latent-Q work per core. K/V and q_new
stay resident in SBUF; only the folded weights round-trip through DRAM.

Each core computes a partial out^T = (attn_out_g @ Wout[rows_g]).T for its
4 heads; the host sums the 4 partials per batch and transposes. bout is
added on-device by the g==0 cores only.
"""
import os
import sys

if "/opt/trn_rl_repo" not in sys.path:
    sys.path.insert(0, "/opt/trn_rl_repo")

import numpy as np

D_MODEL = 2048
Q_LAT = 1536
KV_LAT = 512
NUM_HEADS = 16
HD = 128
B, S = 2, 2048
SCALE = 1.0 / np.sqrt(2.0 * HD)  # 1/16

QT = 512          # query tile width (matmul free dim)
NQT = S // QT     # 4
NC_DM = D_MODEL // 128   # 16 chunks of the model dim
NC_QL = Q_LAT // 128     # 12
NC_KV = KV_LAT // 128    # 4
NKC = S // 128           # 16 key chunks

_CACHE = {}
LAST_RESULT = None


def _strip_cols(g):
    """Global column starts (width 128) of the 4 local head strips, in local
    order [2g, 2g+1, 2g+8, 2g+9]."""
    return [256 * g, 256 * g + 128, 1024 + 256 * g, 1024 + 256 * g + 128]


def _build_bass():
    from concourse import bacc, mybir
    from concourse.tile import TileContext

    f32 = mybir.dt.float32
    bf16 = mybir.dt.bfloat16
    AF = mybir.ActivationFunctionType

    nc = bacc.Bacc("TRN2", target_bir_lowering=False, debug=False, num_devices=8)

    def inp(name, shape, dt=bf16):
        return nc.dram_tensor(name, list(shape), dt, kind="ExternalInput")

    xqT = inp("xqT", (D_MODEL, S))
    xk_sh = inp("xk_sh", (D_MODEL, QT))  # this core's k-tile of xkT
    # Wq_down^T tiles for the fold, c-major, this core's 8 dm-chunks
    wq_downT = inp("wq_downT", (8, 128, NC_QL * 128))
    wkv_down = inp("wkv_down", (NC_KV, 128, NC_DM * 128))  # [s][p=dm][c*128+f]
    wk_rope = inp("wk_rope", (128, NC_DM * 128))           # [p=dm][c*128+f]
    # up-proj slices for the fold: [p=lat][l(lat-chunk)*512 + f(4 strips x 128)]
    wq_up = inp("wq_up", (128, NC_QL * 512))
    wq_rope = inp("wq_rope", (128, NC_QL * 512))
    wk_up = inp("wk_up", (4, 128, NC_KV * 128))            # [strip][p=lat][c*128+f]
    wv_up = inp("wv_up", (128, NC_KV * 512))               # [p=lat][c*512+f]
    wout = inp("wout", (128, 64 * 128))                    # [p][(m*4+h)*128+f]
    cos_q = inp("cos_q", (2, 128, S), f32)                 # [block j][d][q]
    sin_q = inp("sin_q", (2, 128, S), f32)
    cos_k = inp("cos_k", (64, QT))
    sin_k = inp("sin_k", (64, QT))
    masks = inp("masks", (128, 4 * QT))                    # [kl][(o*QT)+ql]
    ones = inp("ones", (128, 128))
    bias = inp("bias", (128, NC_DM), f32)                  # [p][m]

    outT = nc.dram_tensor("outT", [D_MODEL, S], f32, kind="ExternalOutput")

    # folded Weff shards + gathers (2-core batch-pair allgather)
    weff_sh_d = nc.dram_tensor("weff_sh_d", [2, 8, 128, 512], bf16, kind="Internal")
    weff_g_d = nc.dram_tensor("weff_g_d", [2, 2, 8, 128, 512], bf16, kind="Internal")
    # latkv/krope shards + gathers (4-core batch-group allgather; rank = k-tile)
    latkv_sh_d = nc.dram_tensor("latkv_sh_d", [NC_KV, 128, QT], bf16, kind="Internal")
    latkv_g_d = nc.dram_tensor("latkv_g_d", [4, NC_KV, 128, QT], bf16, kind="Internal")
    krope_sh_d = nc.dram_tensor("krope_sh_d", [128, QT], bf16, kind="Internal")
    krope_g_d = nc.dram_tensor("krope_g_d", [4, 128, QT], bf16, kind="Internal")
    G_BATCH = [[0, 1, 2, 3], [4, 5, 6, 7]]
    G_PAIR = [[0, 4], [1, 5], [2, 6], [3, 7]]

    xqT_v = xqT.ap().rearrange("(c p) q -> p c q", p=128)  # [128, 16, 2048]
    xk_sh_v = xk_sh.ap().rearrange("(c p) q -> p c q", p=128)  # [128, 16, 512]

    with TileContext(nc) as tc:
        with tc.tile_pool(name="kvres", bufs=1) as kvres, \
             tc.tile_pool(name="qnres", bufs=1) as qnres, \
             tc.tile_pool(name="xstream", bufs=2) as xstream:
            # resident outputs of phase A1 / A2 (consumed in phase B)
            kproj_sb = kvres.tile([128, 4, S], bf16)
            krope_sb = kvres.tile([128, S], bf16)  # filled from krope_g_d
            v_sb = kvres.tile([128, NKC, 512], bf16)
            qn_sb = qnres.tile([128, 8, S], bf16)  # [2*strip + (0=proj,1=rope)]

            # ----- Phase A1s + F: shard latkv/krope, fold Weff, allgather ---
            with tc.tile_pool(name="a1s", bufs=1) as a1s, \
                 tc.tile_pool(name="a1st", bufs=2) as a1st, \
                 tc.tile_pool(name="pfw", bufs=1) as pfw, \
                 tc.tile_pool(name="pfd", bufs=2) as pfd, \
                 tc.tile_pool(name="pfc", bufs=3) as pfc, \
                 tc.tile_pool(name="a1ps", bufs=2, space="PSUM") as a1ps, \
                 tc.tile_pool(name="pfps", bufs=2, space="PSUM") as pfps:
                # --- latkv + krope for this core's k-tile only ---
                wkv_sb = a1s.tile([128, NC_KV * NC_DM * 128], bf16)
                for s in range(NC_KV):
                    nc.sync.dma_start(
                        out=wkv_sb[:, s * NC_DM * 128:(s + 1) * NC_DM * 128],
                        in_=wkv_down.ap()[s])
                wkr_sb = a1s.tile([128, NC_DM * 128], bf16)
                nc.sync.dma_start(out=wkr_sb, in_=wk_rope.ap())
                cosk_sb = a1s.tile([64, QT], bf16)
                sink_sb = a1s.tile([64, QT], bf16)
                nc.sync.dma_start(out=cosk_sb, in_=cos_k.ap())
                nc.sync.dma_start(out=sink_sb, in_=sin_k.ap())
                xk_t = a1s.tile([128, NC_DM, QT], bf16)
                nc.sync.dma_start(out=xk_t, in_=xk_sh_v)

                latkv_sh = a1s.tile([128, NC_KV, QT], bf16)
                for s in range(NC_KV):
                    ps = a1ps.tile([128, QT], f32, tag="ps")
                    for c in range(NC_DM):
                        nc.tensor.matmul(
                            ps, wkv_sb[:, (s * NC_DM + c) * 128:(s * NC_DM + c + 1) * 128],
                            xk_t[:, c, :], start=(c == 0), stop=(c == NC_DM - 1))
                    nc.scalar.copy(out=latkv_sh[:, s, :], in_=ps)
                nc.sync.dma_start(
                    out=latkv_sh_d.ap().rearrange("s p k -> p s k"), in_=latkv_sh)
                # krope for this k-tile
                ps = a1ps.tile([128, QT], f32, tag="ps")
                for c in range(NC_DM):
                    nc.tensor.matmul(
                        ps, wkr_sb[:, c * 128:(c + 1) * 128],
                        xk_t[:, c, :], start=(c == 0), stop=(c == NC_DM - 1))
                krraw = a1st.tile([128, QT], f32, tag="krraw")
                nc.scalar.copy(out=krraw, in_=ps)
                krb = a1st.tile([64, QT], f32, tag="krb")
                nc.sync.dma_start(out=krb, in_=krraw[64:128, :])
                krsh = a1s.tile([128, QT], bf16)
                t1 = a1st.tile([64, QT], f32, tag="krt1")
                t2 = a1st.tile([64, QT], f32, tag="krt2")
                nc.vector.tensor_mul(t1, krraw[0:64, :], cosk_sb)
                nc.vector.tensor_mul(t2, krb, sink_sb)
                nc.vector.tensor_sub(krsh[0:64, :], t1, t2)
                obot = a1st.tile([64, QT], bf16, tag="krob")
                nc.vector.tensor_mul(t1, krb, cosk_sb)
                nc.vector.tensor_mul(t2, krraw[0:64, :], sink_sb)
                nc.vector.tensor_add(obot, t1, t2)
                nc.sync.dma_start(out=krsh[64:128, :], in_=obot)
                nc.sync.dma_start(out=krope_sh_d.ap(), in_=krsh)

                nc.gpsimd.collective_compute(
                    "AllGather", mybir.AluOpType.bypass, replica_groups=G_BATCH,
                    ins=[latkv_sh_d.ap()], outs=[latkv_g_d.ap()])
                nc.gpsimd.collective_compute(
                    "AllGather", mybir.AluOpType.bypass, replica_groups=G_BATCH,
                    ins=[krope_sh_d.ap()], outs=[krope_g_d.ap()])

                # --- fold this core's 8 dm-chunks of Weff (both kinds) ---
                wqu_sb = pfw.tile([128, NC_QL * 512], bf16)
                wqr_sb = pfw.tile([128, NC_QL * 512], bf16)
                nc.sync.dma_start(out=wqu_sb, in_=wq_up.ap())
                nc.sync.dma_start(out=wqr_sb, in_=wq_rope.ap())
                for c in range(8):
                    wqdT_c = pfd.tile([128, NC_QL, 128], bf16, tag="wqdT")
                    nc.sync.dma_start(out=wqdT_c, in_=wq_downT.ap()[c])
                    for kind, wup in ((0, wqu_sb), (1, wqr_sb)):
                        ps = pfps.tile([128, 512], f32, tag="fps")
                        for l in range(NC_QL):
                            nc.tensor.matmul(
                                ps, wqdT_c[:, l, :], wup[:, l * 512:(l + 1) * 512],
                                start=(l == 0), stop=(l == NC_QL - 1))
                        cp = pfc.tile([128, 512], bf16, tag="cp")
                        nc.scalar.copy(out=cp, in_=ps)
                        nc.sync.dma_start(out=weff_sh_d.ap()[kind][c], in_=cp)
                nc.gpsimd.collective_compute(
                    "AllGather", mybir.AluOpType.bypass, replica_groups=G_PAIR,
                    ins=[weff_sh_d.ap()], outs=[weff_g_d.ap()])

            # ----- Phase A1p: k_proj + V from gathered latkv ----------------
            with tc.tile_pool(name="a1p", bufs=1) as a1p, \
                 tc.tile_pool(name="a1pps", bufs=2, space="PSUM") as a1pps:
                latkv_sb = a1p.tile([128, 4, NC_KV, QT], bf16)
                for kt in range(4):
                    nc.sync.dma_start(
                        out=latkv_sb[:, kt],
                        in_=latkv_g_d.ap()[kt].rearrange("s p k -> p s k"))
                nc.sync.dma_start(
                    out=krope_sb.rearrange("p (t k) -> p t k", t=4),
                    in_=krope_g_d.ap().rearrange("t p k -> p t k"))
                wku_sb = a1p.tile([128, 4 * NC_KV * 128], bf16)
                for s in range(4):
                    nc.sync.dma_start(
                        out=wku_sb[:, s * NC_KV * 128:(s + 1) * NC_KV * 128],
                        in_=wk_up.ap()[s])
                wvu_sb = a1p.tile([128, NC_KV * 512], bf16)
                nc.sync.dma_start(out=wvu_sb, in_=wv_up.ap())
                for kt in range(4):
                    for s in range(4):
                        ps = a1pps.tile([128, QT], f32, tag="ps")
                        for c in range(NC_KV):
                            nc.tensor.matmul(
                                ps, wku_sb[:, (s * NC_KV + c) * 128:(s * NC_KV + c + 1) * 128],
                                latkv_sb[:, kt, c, :], start=(c == 0), stop=(c == NC_KV - 1))
                        nc.scalar.copy(out=kproj_sb[:, s, kt * QT:(kt + 1) * QT], in_=ps)
                    for kc in range(4):
                        ps = a1pps.tile([128, 512], f32, tag="ps")
                        for c in range(NC_KV):
                            nc.tensor.matmul(
                                ps, latkv_sb[:, kt, c, kc * 128:(kc + 1) * 128],
                                wvu_sb[:, c * 512:(c + 1) * 512],
                                start=(c == 0), stop=(c == NC_KV - 1))
                        nc.scalar.copy(out=v_sb[:, kt * 4 + kc, :], in_=ps)

            # ------------- Phase A2: q_proj + q_rope strips from x ---------
            with tc.tile_pool(name="a2w", bufs=1) as a2w, \
                 tc.tile_pool(name="a2t", bufs=2) as a2t, \
                 tc.tile_pool(name="a2ps", bufs=2, space="PSUM") as a2ps:
                weffa_sb = a2w.tile([128, NC_DM, 512], bf16)
                weffb_sb = a2w.tile([128, NC_DM, 512], bf16)
                for rk in range(2):
                    nc.sync.dma_start(
                        out=weffa_sb[:, 8 * rk:8 * rk + 8, :],
                        in_=weff_g_d.ap()[rk][0].rearrange("c p f -> p c f"))
                    nc.sync.dma_start(
                        out=weffb_sb[:, 8 * rk:8 * rk + 8, :],
                        in_=weff_g_d.ap()[rk][1].rearrange("c p f -> p c f"))
                cosq_sb = a2w.tile([128, 2, S], f32)
                sinq_sb = a2w.tile([128, 2, S], f32)
                for j in range(2):
                    nc.sync.dma_start(out=cosq_sb[:, j, :], in_=cos_q.ap()[j])
                    nc.sync.dma_start(out=sinq_sb[:, j, :], in_=sin_q.ap()[j])
                for qt in range(NQT):
                    q0 = qt * QT
                    xq_t = xstream.tile([128, NC_DM, QT], bf16, tag="x")
                    nc.sync.dma_start(out=xq_t, in_=xqT_v[:, :, q0:q0 + QT])
                    for s in range(4):
                        ps = a2ps.tile([128, QT], f32, tag="ps")
                        for c in range(NC_DM):
                            nc.tensor.matmul(
                                ps, weffa_sb[:, c, s * 128:(s + 1) * 128],
                                xq_t[:, c, :], start=(c == 0), stop=(c == NC_DM - 1))
                        nc.scalar.copy(out=qn_sb[:, 2 * s, q0:q0 + QT], in_=ps)
                    raw = []
                    for s in range(4):
                        ps = a2ps.tile([128, QT], f32, tag="ps")
                        for c in range(NC_DM):
                            nc.tensor.matmul(
                                ps, weffb_sb[:, c, s * 128:(s + 1) * 128],
                                xq_t[:, c, :], start=(c == 0), stop=(c == NC_DM - 1))
                        rw = a2t.tile([128, QT], f32, tag=f"raw{s}")
                        nc.scalar.copy(out=rw, in_=ps)
                        raw.append(rw)
                    for j in range(2):
                        a, b = raw[j], raw[2 + j]
                        cj = cosq_sb[:, j, q0:q0 + QT]
                        sj = sinq_sb[:, j, q0:q0 + QT]
                        t1 = a2t.tile([128, QT], f32, tag=f"t1{j}")
                        t2 = a2t.tile([128, QT], f32, tag=f"t2{j}")
                        nc.vector.tensor_mul(t1, a, cj)
                        nc.vector.tensor_mul(t2, b, sj)
                        nc.vector.tensor_sub(qn_sb[:, 2 * j + 1, q0:q0 + QT], t1, t2)
                        nc.vector.tensor_mul(t1, b, cj)
                        nc.vector.tensor_mul(t2, a, sj)
                        nc.vector.tensor_add(qn_sb[:, 2 * (2 + j) + 1, q0:q0 + QT], t1, t2)

            # ------------- Phase B: attention + output projection ----------
            with tc.tile_pool(name="bw", bufs=1) as bw, \
                 tc.tile_pool(name="be", bufs=4) as be, \
                 tc.tile_pool(name="ba", bufs=2) as ba, \
                 tc.tile_pool(name="bo", bufs=2) as bo, \
                 tc.tile_pool(name="bps", bufs=2, space="PSUM") as bps, \
                 tc.tile_pool(name="bpd", bufs=2, space="PSUM") as bpd, \
                 tc.tile_pool(name="bpv", bufs=2, space="PSUM") as bpv, \
                 tc.tile_pool(name="bpo", bufs=2, space="PSUM") as bpo:
                wout_sb = bw.tile([128, 64 * 128], bf16)
                nc.sync.dma_start(out=wout_sb, in_=wout.ap())
                masks_sb = bw.tile([128, 4 * QT], bf16)
                nc.sync.dma_start(out=masks_sb, in_=masks.ap())
                ones_sb = bw.tile([128, 128], bf16)
                nc.sync.dma_start(out=ones_sb, in_=ones.ap())
                bias_sb = bw.tile([128, NC_DM], f32)
                nc.sync.dma_start(out=bias_sb, in_=bias.ap())

                for qt in range(NQT):
                    q0 = qt * QT
                    K = (q0 + QT) // 128  # causal: chunks 0..K-1
                    attn = ba.tile([128, 4, QT], bf16, tag="attn")
                    for h in range(4):
                        psd = bpd.tile([128, QT], f32, tag="psd")
                        psv = bpv.tile([128, QT], f32, tag="psv")
                        for kc in range(K):
                            pss = bps.tile([128, QT], f32, tag="pss")
                            nc.tensor.matmul(
                                pss, kproj_sb[:, h, kc * 128:(kc + 1) * 128],
                                qn_sb[:, 2 * h, q0:q0 + QT], start=True, stop=False)
                            nc.tensor.matmul(
                                pss, krope_sb[:, kc * 128:(kc + 1) * 128],
                                qn_sb[:, 2 * h + 1, q0:q0 + QT], start=False, stop=True)
                            ex = be.tile([128, QT], bf16, tag="ex")
                            nc.scalar.activation(out=ex, in_=pss, func=AF.Exp,
                                                 scale=float(SCALE))
                            o = kc - q0 // 128
                            if o >= 0:  # diagonal chunk: apply causal mask
                                nc.vector.tensor_mul(
                                    ex, ex, masks_sb[:, o * QT:(o + 1) * QT])
                            nc.tensor.matmul(
                                psd, ones_sb, ex,
                                start=(kc == 0), stop=(kc == K - 1),
                                skip_group_check=True)
                            nc.tensor.matmul(
                                psv, v_sb[:, kc, h * 128:(h + 1) * 128], ex,
                                start=(kc == 0), stop=(kc == K - 1),
                                skip_group_check=True)
                        rec = be.tile([128, QT], f32, tag="rec")
                        nc.vector.reciprocal_approx_fast(out=rec, in_=psd)
                        nc.vector.tensor_mul(attn[:, h, :], psv, rec)
                    # output projection for this q tile
                    for m in range(NC_DM):
                        pso = bpo.tile([128, QT], f32, tag="pso")
                        for h in range(4):
                            nc.tensor.matmul(
                                pso, wout_sb[:, (m * 4 + h) * 128:(m * 4 + h + 1) * 128],
                                attn[:, h, :], start=(h == 0), stop=(h == 3))
                        oc = bo.tile([128, QT], f32, tag="oc")
                        nc.scalar.activation(
                            out=oc, in_=pso, func=AF.Identity,
                            bias=bias_sb[:, m:m + 1], scale=1.0)
                        nc.sync.dma_start(
                            out=outT.ap()[m * 128:(m + 1) * 128, q0:q0 + QT], in_=oc)

    nc.finalize()
    return nc


def _host_pack(inputs):
    """Build the 8 per-core input maps from the full inputs."""
    import ml_dtypes
    bf16 = ml_dtypes.bfloat16

    xq = np.ascontiguousarray(inputs["inputs_q"], dtype=np.float32)
    xk = np.ascontiguousarray(inputs["inputs_k"], dtype=np.float32)
    Wq_down = np.asarray(inputs["Wq_down"], dtype=np.float32)
    Wkv_down = np.asarray(inputs["Wkv_down"], dtype=np.float32)
    Wq_up = np.asarray(inputs["Wq_up"], dtype=np.float32)
    Wk_up = np.asarray(inputs["Wk_up"], dtype=np.float32)
    Wv_up = np.asarray(inputs["Wv_up"], dtype=np.float32)
    Wq_rope = np.asarray(inputs["Wq_rope"], dtype=np.float32)
    Wk_rope = np.asarray(inputs["Wk_rope"], dtype=np.float32)
    Wout = np.asarray(inputs["Wout"], dtype=np.float32)
    bout = np.asarray(inputs["bout"], dtype=np.float32)

    def pack_lhs(W, n_strips, strip_starts, nchunks):
        # -> [n_strips, 128, nchunks*128]: [s][p][c*128+f]
        out = np.empty((n_strips, 128, nchunks * 128), dtype=bf16)
        for s in range(n_strips):
            blk = W[:, strip_starts[s]:strip_starts[s] + 128]  # [nchunks*128, 128]
            out[s] = blk.reshape(nchunks, 128, 128).transpose(1, 0, 2).reshape(128, -1).astype(bf16)
        return out

    xqT = [np.ascontiguousarray(xq[b].T.astype(bf16)) for b in range(B)]
    xkT = [xk[b].T.astype(bf16) for b in range(B)]

    # Wq_down^T packed c-major for the fold: [c][p=lat][l*128+f(dm)]
    WqdT = np.ascontiguousarray(Wq_down.T)  # [Q_LAT, D_MODEL]
    wq_downT_full = np.ascontiguousarray(
        WqdT.reshape(NC_QL, 128, NC_DM, 128).transpose(2, 1, 0, 3)
        .reshape(NC_DM, 128, NC_QL * 128).astype(bf16))

    wkv_down_p = pack_lhs(Wkv_down, NC_KV, [128 * s for s in range(NC_KV)], NC_DM)
    wk_rope_p = pack_lhs(Wk_rope, 1, [0], NC_DM)[0]

    # rope tables (fp32)
    iq = np.arange(1024, dtype=np.float64)
    inv_q = 1.0 / (10000.0 ** (iq * 2.0 / D_MODEL))
    pos = np.arange(S, dtype=np.float64)
    ang_q = pos[:, None] * inv_q[None, :]          # [S, 1024]
    ik = np.arange(64, dtype=np.float64)
    inv_k = 1.0 / (10000.0 ** (ik * 2.0 / HD))
    ang_k = pos[:, None] * inv_k[None, :]          # [S, 64]
    cos_k_full = np.cos(ang_k).T.astype(bf16)  # [64, S]
    sin_k_full = np.sin(ang_k).T.astype(bf16)

    kl = np.arange(128)[:, None]
    ql = np.arange(QT)[None, :]
    masks = np.concatenate(
        [(kl + 128 * o <= ql).astype(np.float32) for o in range(4)], axis=1)
    masks = np.ascontiguousarray(masks.astype(bf16))
    ones = np.ones((128, 128), dtype=bf16)

    in_maps = []
    for c in range(8):
        b, g = divmod(c, 4)
        cols = _strip_cols(g)
        cols4 = np.concatenate([np.arange(cs, cs + 128) for cs in cols])

        # up-proj slices for fold rhs: [p=lat within chunk][l*512 + f]
        def pack_up(W):
            Wg = W[:, cols4]  # [Q_LAT, 512]
            return np.ascontiguousarray(
                Wg.reshape(NC_QL, 128, 512).transpose(1, 0, 2).reshape(128, -1).astype(bf16))
        wq_up_p = pack_up(Wq_up)
        wq_rope_p = pack_up(Wq_rope)
        wk_up_p = pack_lhs(Wk_up, 4, cols, NC_KV)
        Wv_g = Wv_up[:, cols4]                      # [512, 512]
        wv_up_p = np.ascontiguousarray(
            Wv_g.reshape(NC_KV, 128, 512).transpose(1, 0, 2).reshape(128, -1).astype(bf16))
        Wout_g = Wout[cols4, :].reshape(4, 128, NC_DM, 128)   # [h][p][m][f]
        wout_p = np.ascontiguousarray(
            Wout_g.transpose(1, 2, 0, 3).reshape(128, -1).astype(bf16))
        cos_q_p = np.empty((2, 128, S), dtype=np.float32)
        sin_q_p = np.empty((2, 128, S), dtype=np.float32)
        for j in range(2):
            idx = 256 * g + 128 * j + np.arange(128)
            cos_q_p[j] = np.cos(ang_q[:, idx]).T
            sin_q_p[j] = np.sin(ang_q[:, idx]).T
        bias_p = (bout if g == 0 else np.zeros_like(bout)).reshape(NC_DM, 128)
        bias_p = np.ascontiguousarray(bias_p.T)     # [128, m]

        k0 = QT * g
        in_maps.append({
            "xqT": xqT[b],
            "xk_sh": np.ascontiguousarray(xkT[b][:, k0:k0 + QT]),
            "wq_downT": np.ascontiguousarray(wq_downT_full[8 * b:8 * b + 8]),
            "wkv_down": wkv_down_p, "wk_rope": wk_rope_p,
            "wq_up": wq_up_p, "wq_rope": wq_rope_p, "wk_up": wk_up_p,
            "wv_up": wv_up_p, "wout": wout_p,
            "cos_q": cos_q_p, "sin_q": sin_q_p,
            "cos_k": np.ascontiguousarray(cos_k_full[:, k0:k0 + QT]),
            "sin_k": np.ascontiguousarray(sin_k_full[:, k0:k0 + QT]),
            "masks": masks, "ones": ones, "bias": bias_p,
        })
    return in_maps


def kernel(**inputs):
    global LAST_RESULT
    from concourse.bass_utils import run_bass_kernel_spmd

    if "nc" not in _CACHE:
        _CACHE["nc"] = _build_bass()
    nc = _CACHE["nc"]

    in_maps = _host_pack(inputs)
    kwargs = {}
    if os.environ.get("KERNEL_TRACE"):
        try:
            sys.path.insert(0, os.path.dirname(os.path.abspath(__file__)))
            import axon_shim
            axon_shim.install()
        except Exception:
            pass
        kwargs["trace"] = True
    res = run_bass_kernel_spmd(nc, in_maps, core_ids=list(range(8)), **kwargs)
    LAST_RESULT = res

    out = np.empty((B, S, D_MODEL), dtype=np.float32)
    for b in range(B):
        acc = res.results[4 * b]["outT"].copy()
        for g in range(1, 4):
            acc += res.results[4 * b + g]["outT"]
        out[b] = acc.T
    return out
